# revision 4
# baseline (speedup 1.0000x reference)
"""Trainium2 Bass kernel for nn_Condensation: 10 sequential masked-Gaussian-blur
composites over a [16,3,768,768] image, data-parallel over 8 NeuronCores.

v4 strategy (per core, 2 images = 6 image-channels):
  - Row-offset block grid (delta chosen so EVERY drop's mask support fits in
    exactly 2 h-blocks of 128 rows). Cuts elementwise/mask/matmul work ~25%
    vs a 0-based grid (where 5 drops straddled 3 blocks) and removes the
    false inter-drop dependencies block padding created.
  - Drops emitted in exact topological order of their true spatial overlap
    DAG (non-overlapping drops commute): 3 levels of 4/4/2 drops in flight
    instead of 5 waves of 2 -> much better engine ILP.
  - State resident in SBUF as bf16 [128, 6, NB, 768]; only the 416 rows any
    drop touches are loaded/stored (partial-partition edge blocks, zero-
    padded loads); host copies untouched rows.
  - Separable blur as two banded-matmul passes on TensorE (bf16, f32 PSUM),
    support-clipped bands; q-trick composite (q = out - om) with per-op
    greedy balancing across Vector/Scalar/GpSimd using trace-calibrated
    costs (V bf16 2x-mode vs PSUM 1x, S copy-only, G slow).
  - Stores split per (block, w-piece) keyed to each piece's LAST writer in
    emission order, so most output DMA drains long before the final drop.
"""
import numpy as np
import ml_dtypes

NUM_DROPS = 10
MIN_R, MAX_R = 60.0, 80.0
BETA = 1.8
BLUR_RADII = [11.3535, 17.9381, 5.7966, 10.8586, 5.5301, 15.9075, 12.3225, 13.4871, 6.6639, 9.5413]


def _ksize(r):
    k = int(2 * r) + 1
    return k + 1 if k % 2 == 0 else k


KSIZES = [_ksize(r) for r in BLUR_RADII]
H = W = 768
B_TOTAL, C = 16, 3
N_CORES = 8
B_LOC = B_TOTAL // N_CORES          # 2 images per core
IC = B_LOC * C                      # 6 image-channels per core
NG = IC // 2                        # 3 pairs of image-channels
P = 128
EPS = 5e-3                          # mask support threshold (error-validated)

_bf16 = ml_dtypes.bfloat16


def _conv_matrix(sigma, ksize, n=768):
    """n x n matrix Kmat with blur_1d(x) = Kmat @ x, matching the reference
    (correlation with normalized gaussian, 'reflect' padding)."""
    half = (ksize - 1) * 0.5
    xs = np.linspace(-half, half, ksize)
    pdf = np.exp(-0.5 * (xs / np.float64(sigma)) ** 2)
    k1 = (pdf / pdf.sum()).astype(np.float32).astype(np.float64)
    pad = ksize // 2
    Kmat = np.zeros((n, n), dtype=np.float64)
    idx = np.arange(n)[:, None] + np.arange(ksize)[None, :] - pad
    idx = np.abs(idx)
    idx = np.where(idx >= n, 2 * n - 2 - idx, idx)
    np.add.at(Kmat, (np.repeat(np.arange(n), ksize), idx.ravel()),
              np.tile(k1, n))
    return Kmat.astype(np.float32)


class _Drop:
    pass


def _drop_meta(positions, radius):
    """Host-side per-drop geometry + tensors (shared across cores) on the
    row-offset block grid."""
    pos = np.clip(np.asarray(positions, np.float32), -1.0, 1.0)
    rad = np.clip(np.asarray(radius, np.float32), MIN_R, MAX_R)
    s = float(np.sqrt((-np.log(EPS)) ** (1.0 / BETA)))
    s2 = s * s

    geo = []
    for j in range(NUM_DROPS):
        x0 = (pos[j, 0] + 1.0) / 2.0 * W
        y0 = (pos[j, 1] + 1.0) / 2.0 * H
        wr = rad[j]
        hr = wr * np.float32(0.8)
        p = KSIZES[j] // 2
        h0 = max(0, int(np.floor(y0 - s * hr))) & ~1
        h1 = min(H, (int(np.ceil(y0 + s * hr)) + 2) & ~1)
        w0 = max(0, int(np.floor(x0 - s * wr))) & ~1
        w1 = min(W, (int(np.ceil(x0 + s * wr)) + 2) & ~1)
        geo.append([h0, h1, w0, w1, p, float(x0), float(y0), float(wr), float(hr)])

    # pick an even grid offset so every drop spans exactly 2 blocks
    delta = None
    for dd_ in range(0, 128, 2):
        if all(((g[0] - dd_) % 128) + (g[1] - g[0]) <= 256 for g in geo):
            delta = dd_
            break
    assert delta is not None, "no 2-block grid offset exists"
    hmin = min(g[0] for g in geo)
    hmax = max(g[1] for g in geo)
    g0 = hmin - ((hmin - delta) % 128)
    NB = -((g0 - hmax) // 128)

    drops = []
    for j in range(NUM_DROPS):
        h0, h1, w0, w1, p, x0, y0, wr, hr = geo[j]
        d = _Drop()
        d.j, d.p = j, p
        d.B0 = (h0 - g0) // 128
        d.HBs = g0 + 128 * d.B0
        assert h1 - d.HBs <= 256 and d.B0 + 2 <= NB
        # cap w so Wt <= 256 (two overlapping 128-col chunks)
        wcap = 256 - 2 * p - 2
        while w1 - w0 > wcap:
            if x0 - w0 > w1 - x0:
                w0 += 2
            else:
                w1 -= 2
        d.h0, d.h1, d.w0, d.w1 = h0, h1, w0, w1
        d.span = h1 - h0
        d.Wr = w1 - w0
        d.voff = h0 - d.HBs
        wa = max(0, w0 - p) & ~1
        wb = min(W, (w1 + p + 1) & ~1)
        d.wa, d.wb = wa, wb
        d.Wt = wb - wa
        assert d.Wt <= 256 and d.span <= 256
        d.WBn = (d.Wt + P - 1) // P
        assert d.WBn == 2
        d.cstarts = [wa, wb - P]

        # pass A bands per k-block: output h' range (relative to h0)
        d.bandsA = []
        for k in range(2):
            a = max(0, d.HBs + P * k - p - h0)
            b = min(d.span, d.HBs + P * (k + 1) + p - h0)
            d.bandsA.append((a, b))

        # per h-block composite w-range [wl, wr) from the ellipse extent
        d.hbw = []
        for hb in range(2):
            ra = max(h0, d.HBs + P * hb)
            rb = min(h1, d.HBs + P * (hb + 1))
            if ra - 1 < y0 < rb:
                dh = 0.0
            else:
                dh = min(abs(ra - y0), abs(rb - 1 - y0))
            half = wr * np.sqrt(max(0.0, s2 - (dh / hr) ** 2))
            wl = max(w0, (int(np.floor(x0 - half)) - 2) & ~1)
            wr_ = min(w1, (int(np.ceil(x0 + half)) + 4) & ~1)
            wr_ = max(wr_, wl + 2)
            d.hbw.append((wl, wr_))

        # pass B bands per (hb, wc): output w' range (relative w0), or None
        d.bandsB = []
        for hb in range(2):
            wl, wr_ = d.hbw[hb]
            row = []
            for wc in range(2):
                c = d.cstarts[wc]
                a = max(wl, c - p)
                b = min(wr_, c + P + p)
                row.append((a - w0, b - w0) if b > a else None)
            d.bandsB.append(row)

        # mask over [2 blocks of 128 rows] x [wa:wb], zero outside support
        rows = (d.HBs + np.arange(2 * P, dtype=np.int64)).astype(np.float32)
        dd = (rows[:, None] - y0) ** 2 / hr ** 2 + \
             (np.arange(wa, wb, dtype=np.float32)[None, :] - x0) ** 2 / wr ** 2
        m = np.clip(np.exp(-(dd.astype(np.float32) ** np.float32(BETA)) + np.float32(1e-10)), 0.0, 1.0)
        m = np.where(dd <= np.float32(s2), m, 0.0).astype(np.float32)
        mz = np.zeros_like(m)
        for hb in range(2):
            ra = max(h0, d.HBs + P * hb) - d.HBs
            rb = min(h1, d.HBs + P * (hb + 1)) - d.HBs
            wl, wr_ = d.hbw[hb]
            mz[ra:rb, wl - wa:wr_ - wa] = m[ra:rb, wl - wa:wr_ - wa]
        m1 = np.ascontiguousarray(
            mz.reshape(2, P, d.Wt).transpose(1, 0, 2)).astype(_bf16)
        d.m_np = np.ascontiguousarray(
            np.broadcast_to(m1[:, None], (P, 2, 2, d.Wt)))

        MT = _conv_matrix(BLUR_RADII[j], KSIZES[j]).T    # MT[src, dst]
        kv = np.zeros((P, 2, d.span), np.float32)
        for k in range(2):
            r0 = d.HBs + P * k
            lo = max(0, -r0)
            hi = min(P, H - r0)
            if hi > lo:
                kv[lo:hi, k, :] = MT[r0 + lo:r0 + hi, h0:h1]
        d.kv_np = np.ascontiguousarray(kv.astype(_bf16))
        kh = np.zeros((P, 2, d.Wr), np.float32)
        for wc in range(2):
            c = d.cstarts[wc]
            kh[:, wc, :] = MT[c:c + P, w0:w1]
        # the second w-chunk overlaps the first: zero duplicated rows
        dup = d.cstarts[0] + P - d.cstarts[1]
        if dup > 0:
            kh[:dup, 1, :] = 0.0
        d.kh_np = np.ascontiguousarray(kh.astype(_bf16))
        drops.append(d)
    return drops, g0, NB, hmin, hmax


def _topo_order(drops):
    """Exact dependency DAG on (block-range x w-range) slice overlap;
    emission order = stable topological levels."""
    def _dep(i, j):
        di, dj_ = drops[i], drops[j]
        if abs(di.B0 - dj_.B0) > 1:
            return False
        ri, wi = (di.wa, di.wb), (di.w0, di.w1)
        rj, wj = (dj_.wa, dj_.wb), (dj_.w0, dj_.w1)
        for (a, b) in ((wi, rj), (ri, wj), (wi, wj)):
            if max(a[0], b[0]) < min(a[1], b[1]):
                return True
        return False

    level = [0] * NUM_DROPS
    for j in range(NUM_DROPS):
        for i in range(j):
            if _dep(i, j):
                level[j] = max(level[j], level[i] + 1)
    order = sorted(range(NUM_DROPS), key=lambda j: (level[j], j))
    return order, level


def _store_pieces(drops, order, NB):
    """Per block: split [0,W) into up to 3 w-pieces, each tagged with the
    emission position of its LAST writer (-1 = never written)."""
    pieces = {}
    for blk in range(NB):
        last = np.full(W, -1, np.int64)
        for pos, dj in enumerate(order):
            d = drops[dj]
            if d.B0 <= blk <= d.B0 + 1:
                last[d.w0:d.w1] = pos
        runs = []
        ws = 0
        for x in range(1, W + 1):
            if x == W or last[x] != last[ws]:
                runs.append([ws, x, int(last[ws])])
                ws = x
        # merge small runs / cap count; merged run stores after max(pos)
        def _merge_once():
            k = min(range(len(runs)), key=lambda i: runs[i][1] - runs[i][0])
            if k == 0:
                k2 = 1
            elif k == len(runs) - 1:
                k2 = k - 1
            else:
                k2 = k - 1 if (runs[k - 1][1] - runs[k - 1][0]) < (runs[k + 1][1] - runs[k + 1][0]) else k + 1
            a, b = min(k, k2), max(k, k2)
            runs[a] = [runs[a][0], runs[b][1], max(runs[a][2], runs[b][2])]
            del runs[b]
        while len(runs) > 3 or min(r[1] - r[0] for r in runs) < 96:
            _merge_once()
        # even alignment
        for r in runs:
            r[0] &= ~1
        for i in range(len(runs) - 1):
            runs[i][1] = runs[i + 1][0]
        runs[-1][1] = W
        pieces[blk] = [(r[0], r[1], r[2]) for r in runs]
    return pieces


class _Balancer:
    """Greedy static load-balancer across Vector/Scalar/GpSimd with
    trace-calibrated per-op costs (ns). V gets 2x DVE mode on all-SBUF bf16
    ops; PSUM-source ops run 1x. S (Activation) can only copy."""

    def __init__(self, nc):
        self.nc = nc
        self.load = {'V': 0.0, 'S': 0.0, 'G': 0.0}

    def _pick(self, costs):
        eng, c = min(costs, key=lambda ec: self.load[ec[0]] + ec[1])
        self.load[eng] += c
        return eng

    def tt(self, op, out, a, b, fd):
        # all-SBUF bf16 tensor_tensor (GPSIMD cannot touch PSUM)
        costs = [('V', fd * 0.55 + 130), ('G', fd * 1.3 + 190)]
        eng = self._pick(costs)
        e = self.nc.vector if eng == 'V' else self.nc.gpsimd
        getattr(e, 'tensor_' + op)(out, a, b)

    def copy(self, out, src, fd):
        # PSUM f32 -> SBUF bf16 (V at 1x psum rate, S activation copy)
        eng = self._pick([('V', fd * 1.05 + 200), ('S', fd * 0.84 + 170)])
        if eng == 'V':
            self.nc.vector.tensor_copy(out, src)
        else:
            self.nc.scalar.copy(out=out, in_=src)

    def bsh_mul(self, psb_sl, bshp, m_sl, t2_sl, fd, shape, dt):
        """t2 = m * psb, either via {S|V} psum-copy + {V|G} bf16 mul, or
        V direct mul from PSUM."""
        cV, cS = fd * 1.05 + 200, fd * 0.84 + 170
        mV, mG = fd * 0.55 + 130, fd * 1.3 + 190
        dV = fd * 1.05 + 200
        best, opt = None, None
        for tag, deltas in [('SV', (('S', cS), ('V', mV))),
                            ('SG', (('S', cS), ('G', mG))),
                            ('VG', (('V', cV), ('G', mG))),
                            ('D', (('V', dV),))]:
            tmp = dict(self.load)
            for e, c in deltas:
                tmp[e] += c
            key = (max(tmp.values()), sum(tmp.values()))
            if best is None or key < best:
                best, opt = key, (tag, deltas)
        tag, deltas = opt
        for e, c in deltas:
            self.load[e] += c
        if tag == 'D':
            self.nc.vector.tensor_mul(t2_sl, m_sl, psb_sl)
        else:
            bsh = bshp.tile(shape, dt, tag="Bs")
            bsh_sl = bsh[:, :, 0:psb_sl.shape[-1]]
            if tag[0] == 'S':
                self.nc.scalar.copy(out=bsh_sl, in_=psb_sl)
            else:
                self.nc.vector.tensor_copy(bsh_sl, psb_sl)
            e = self.nc.vector if tag[1] == 'V' else self.nc.gpsimd
            e.tensor_mul(t2_sl, m_sl, bsh_sl)


def _build_program(drops, g0, NB, hmin, hmax, order, pieces):
    from contextlib import ExitStack
    from concourse import bacc, tile, mybir

    f32 = mybir.dt.float32
    bf16 = mybir.dt.bfloat16

    nc = bacc.Bacc("TRN2", target_bir_lowering=False, debug=False,
                   num_devices=N_CORES)

    # input params: per (pair, blk), always 128 partitions (host zero-pads
    # rows outside [hmin, hmax))
    imgs_d = [[nc.declare_dram_parameter(f"i{g}b{blk}", [P, 2, W], bf16, False)
               for blk in range(NB)] for g in range(NG)]
    # output params: per (pair, blk, piece), partial partitions on edge blocks
    pu = {blk: (max(0, hmin - (g0 + 128 * blk)),
                min(P, hmax - (g0 + 128 * blk))) for blk in range(NB)}
    outs_d = {}
    for g in range(NG):
        for blk in range(NB):
            p0, p1 = pu[blk]
            for pi, (ws, we, _pos) in enumerate(pieces[blk]):
                outs_d[(g, blk, pi)] = nc.declare_dram_parameter(
                    f"o{g}b{blk}p{pi}", [p1 - p0, 2, we - ws], bf16, True)

    # drop params batched into 3 chunks by emission order
    chunks = [order[0:2], order[2:4], order[4:]]
    kvoffs, khoffs = {}, {}
    kvlen = [0] * len(chunks)
    khlen = [0] * len(chunks)
    for ci, ch in enumerate(chunks):
        for dj in ch:
            d = drops[dj]
            kvoffs[dj] = (ci, kvlen[ci])
            khoffs[dj] = (ci, khlen[ci])
            kvlen[ci] += 2 * d.span
            khlen[ci] += 2 * d.Wr
    WMAX = 256
    pchunks = []
    for ci, ch in enumerate(chunks):
        pchunks.append((
            nc.declare_dram_parameter(f"mc{ci}", [P, len(ch), 2, 2, WMAX], bf16, False),
            nc.declare_dram_parameter(f"kvc{ci}", [P, kvlen[ci]], bf16, False),
            nc.declare_dram_parameter(f"khc{ci}", [P, khlen[ci]], bf16, False)))

    bal = _Balancer(nc)

    with tile.TileContext(nc) as tc, ExitStack() as ctx:
        outp = ctx.enter_context(tc.tile_pool(name="out_state", bufs=1))
        out_s = outp.tile([P, IC, NB, W], bf16, name="state", tag="state")
        dp = ctx.enter_context(tc.tile_pool(name="dropin", bufs=1))
        omp = ctx.enter_context(tc.tile_pool(name="omq", bufs=8))
        vtp = ctx.enter_context(tc.tile_pool(name="vts", bufs=8))
        bshp = ctx.enter_context(tc.tile_pool(name="bsh", bufs=8))
        ppa = ctx.enter_context(tc.tile_pool(name="psa", bufs=2, space="PSUM"))
        ppb = ctx.enter_context(tc.tile_pool(name="psb", bufs=4, space="PSUM"))

        # ---- PE warm-up: matmuls on a zeroed tile span the load window
        wt = dp.tile([P, 512], bf16, tag="warm")
        nc.gpsimd.memset(wt[:], 0)
        warm = ppa.tile([P, 2, 2, 256], f32, tag="psa")
        for i in range(46):
            nc.tensor.matmul(warm[:, 0, 0, 0:256], lhsT=wt[:, 0:P],
                             rhs=wt[:, 0:256], start=True, stop=True)
        # pre-zero the vt ring so pass-B stationaries never read NaN garbage
        for i in range(8):
            v0 = vtp.tile([P, 2, 2, 256], bf16, tag="vt", bufs=8)
            (nc.vector if i % 2 else nc.gpsimd).memset(v0[:], 0)

        # ---- loads: params chunk0 on scalar; imgs pair-major on sync so
        # each drop chain starts as its blocks arrive; later chunks follow
        ptiles = []
        for ci, ch in enumerate(chunks):
            ptiles.append((
                dp.tile([P, len(ch), 2, 2, WMAX], bf16, tag=f"mc{ci}", name=f"mc{ci}"),
                dp.tile([P, kvlen[ci]], bf16, tag=f"kvc{ci}", name=f"kvc{ci}"),
                dp.tile([P, khlen[ci]], bf16, tag=f"khc{ci}", name=f"khc{ci}")))
        for t, pd in zip(ptiles[0], pchunks[0]):
            nc.scalar.dma_start(out=t[:], in_=pd.ap()[:])
        for g in range(NG):
            for blk in range(NB):
                nc.sync.dma_start(out=out_s[:, 2 * g:2 * g + 2, blk, :],
                                  in_=imgs_d[g][blk].ap()[:])
        for ci in (1, 2):
            for t, pd in zip(ptiles[ci], pchunks[ci]):
                nc.scalar.dma_start(out=t[:], in_=pd.ap()[:])

        # position of each drop in emission order, for store scheduling
        pos_of = {dj: pos for pos, dj in enumerate(order)}
        store_after = {}
        for blk in range(NB):
            for pi, (ws, we, pos) in enumerate(pieces[blk]):
                store_after.setdefault(max(pos, 0), []).append((blk, pi, ws, we))

        # ---- drops in topological order
        for pos, dj in enumerate(order):
            d = drops[dj]
            ci = next(i for i, ch in enumerate(chunks) if dj in ch)
            i = chunks[ci].index(dj)
            mt, kvt, kht = ptiles[ci]
            _, kvo = kvoffs[dj]
            _, kho = khoffs[dj]
            for g in range(NG):
                sl = out_s[:, 2 * g:2 * g + 2, d.B0:d.B0 + 2, d.wa:d.wb]
                slq = out_s[:, 2 * g:2 * g + 2, d.B0:d.B0 + 2, d.w0:d.w1]
                om = omp.tile([P, 2, 2, 256], bf16, tag="om")
                bal.tt('mul', om[:, :, :, 0:d.Wt],
                       mt[:, i, 0:2, 0:2, 0:d.Wt], sl, 4 * d.Wt)
                q = omp.tile([P, 2, 2, 256], bf16, tag="q")
                bal.tt('sub', q[:, :, :, 0:d.Wr], slq,
                       om[:, :, :, d.w0 - d.wa:d.w0 - d.wa + d.Wr], 4 * d.Wr)
                # pass A: vT[w-chunk, h'] banded over the support
                psa = ppa.tile([P, 2, 2, 256], f32, tag="psa")
                for wc in range(2):
                    coff = d.cstarts[wc] - d.wa
                    for jj in range(2):
                        for k in range(2):
                            a, b = d.bandsA[k]
                            nc.tensor.matmul(
                                psa[:, jj, wc, a:b],
                                lhsT=om[:, jj, k, coff:coff + P],
                                rhs=kvt[:, kvo + k * d.span + a:kvo + k * d.span + b],
                                start=(k == 0), stop=(k == 1))
                vt = vtp.tile([P, 2, 2, 256], bf16, tag="vt", bufs=8)
                bal.copy(vt[:, :, :, d.voff:d.voff + d.span],
                         psa[:, :, :, 0:d.span], 4 * d.span)
                # pass B + composite per h'-block
                for hb in range(2):
                    psb = ppb.tile([P, 2, 256], f32, tag="psb")
                    live = [(wc, ab) for wc, ab in enumerate(d.bandsB[hb]) if ab]
                    for jj in range(2):
                        for li, (wc, (a, b)) in enumerate(live):
                            nc.tensor.matmul(
                                psb[:, jj, a:b],
                                lhsT=vt[:, jj, wc, hb * P:(hb + 1) * P],
                                rhs=kht[:, kho + wc * d.Wr + a:kho + wc * d.Wr + b],
                                start=(li == 0), stop=(li == len(live) - 1))
                    wl, wr_ = d.hbw[hb]
                    wid = wr_ - wl
                    acol = wl - d.w0
                    t2 = bshp.tile([P, 2, 256], bf16, tag="t2")
                    bal.bsh_mul(psb[:, :, acol:acol + wid], bshp,
                                mt[:, i, 0:2, hb, wl - d.wa:wr_ - d.wa],
                                t2[:, :, 0:wid], 2 * wid, [P, 2, 256], bf16)
                    osl = out_s[:, 2 * g:2 * g + 2, d.B0 + hb, wl:wr_]
                    bal.tt('add', osl,
                           q[:, :, hb, wl - d.w0:wr_ - d.w0],
                           t2[:, :, 0:wid], 2 * wid)
            # stores whose last writer just finished (all pairs emitted)
            for (blk, pi, ws, we) in store_after.get(pos, []):
                p0, p1 = pu[blk]
                for g in range(NG):
                    nc.sync.dma_start(
                        out=outs_d[(g, blk, pi)].ap()[:],
                        in_=out_s[p0:p1, 2 * g:2 * g + 2, blk, ws:we])

    nc.compile()
    print("balancer loads (us):",
          {k: round(v / 1000, 1) for k, v in bal.load.items()})
    return nc


_CACHE = {}


def _get_program(positions, radius):
    key = (np.asarray(positions, np.float32).tobytes(),
           np.asarray(radius, np.float32).tobytes())
    if key not in _CACHE:
        drops, g0, NB, hmin, hmax = _drop_meta(positions, radius)
        order, level = _topo_order(drops)
        print("emission order:", order, "levels:", level)
        pieces = _store_pieces(drops, order, NB)
        nc = _build_program(drops, g0, NB, hmin, hmax, order, pieces)
        _CACHE[key] = (nc, drops, g0, NB, hmin, hmax, order, pieces)
    return _CACHE[key]


def kernel(img, positions, radius, _want_trace=False, **_kw):
    from concourse.bass_utils import run_bass_kernel_spmd
    img = np.asarray(img, np.float32)
    assert img.shape == (B_TOTAL, C, H, W)
    nc, drops, g0, NB, hmin, hmax, order, pieces = _get_program(positions, radius)

    # pack rows [g0, g0+NB*128) to [p, pair(2), w] per (core, pair, blk), bf16,
    # zero-padded outside [hmin, hmax)
    rows_lo, rows_hi = hmin, hmax
    imgb = np.zeros((N_CORES, IC, NB * P, W), _bf16)
    src = img.reshape(N_CORES, IC, H, W)
    imgb[:, :, rows_lo - g0:rows_hi - g0, :] = src[:, :, rows_lo:rows_hi, :].astype(_bf16)
    packed = np.ascontiguousarray(
        imgb.reshape(N_CORES, IC, NB, P, W).transpose(0, 3, 1, 2, 4))

    chunks = [order[0:2], order[2:4], order[4:]]
    WMAX = 256
    base = {}
    for ci, ch in enumerate(chunks):
        mc = np.zeros((P, len(ch), 2, 2, WMAX), _bf16)
        for i, dj in enumerate(ch):
            d = drops[dj]
            mc[:, i, :, :, 0:d.Wt] = d.m_np
        base[f"mc{ci}"] = mc
        base[f"kvc{ci}"] = np.ascontiguousarray(np.concatenate(
            [drops[dj].kv_np.reshape(P, -1) for dj in ch], axis=1))
        base[f"khc{ci}"] = np.ascontiguousarray(np.concatenate(
            [drops[dj].kh_np.reshape(P, -1) for dj in ch], axis=1))
    in_maps = []
    for i in range(N_CORES):
        mp = dict(base)
        for g in range(NG):
            for blk in range(NB):
                mp[f"i{g}b{blk}"] = np.ascontiguousarray(
                    packed[i][:, 2 * g:2 * g + 2, blk, :])
        in_maps.append(mp)
    res = run_bass_kernel_spmd(nc, in_maps, core_ids=list(range(N_CORES)),
                               trace=_want_trace)
    out = img.copy()
    pu = {blk: (max(0, hmin - (g0 + 128 * blk)),
                min(P, hmax - (g0 + 128 * blk))) for blk in range(NB)}
    for i in range(N_CORES):
        oc = out.reshape(N_CORES, IC, H, W)
        for g in range(NG):
            for blk in range(NB):
                p0, p1 = pu[blk]
                r0 = g0 + 128 * blk + p0
                for pi, (ws, we, _pos) in enumerate(pieces[blk]):
                    blkres = res.results[i][f"o{g}b{blk}p{pi}"]
                    # [Pu, 2, wlen] -> rows r0..r0+Pu
                    oc[i, 2 * g:2 * g + 2, r0:r0 + (p1 - p0), ws:we] = \
                        blkres.transpose(1, 0, 2).astype(np.float32)
    if _want_trace:
        return out, res
    return out


# revision 21
# speedup vs baseline: 1.0492x; 1.0492x over previous
"""Trainium2 Bass kernel for nn_Condensation: 10 sequential masked-Gaussian-blur
composites over a [16,3,768,768] image, data-parallel over 8 NeuronCores.

v4 strategy (per core, 2 images = 6 image-channels):
  - Row-offset block grid (delta chosen so EVERY drop's mask support fits in
    exactly 2 h-blocks of 128 rows). Cuts elementwise/mask/matmul work ~25%
    vs a 0-based grid (where 5 drops straddled 3 blocks) and removes the
    false inter-drop dependencies block padding created.
  - Drops emitted in exact topological order of their true spatial overlap
    DAG (non-overlapping drops commute): 3 levels of 4/4/2 drops in flight
    instead of 5 waves of 2 -> much better engine ILP.
  - State resident in SBUF as bf16 [128, 6, NB, 768]; only the 416 rows any
    drop touches are loaded/stored (partial-partition edge blocks, zero-
    padded loads); host copies untouched rows.
  - Separable blur as two banded-matmul passes on TensorE (bf16, f32 PSUM),
    support-clipped bands; q-trick composite (q = out - om) with per-op
    greedy balancing across Vector/Scalar/GpSimd using trace-calibrated
    costs (V bf16 2x-mode vs PSUM 1x, S copy-only, G slow).
  - Stores split per (block, w-piece) keyed to each piece's LAST writer in
    emission order, so most output DMA drains long before the final drop.
"""
import numpy as np
import ml_dtypes

NUM_DROPS = 10
MIN_R, MAX_R = 60.0, 80.0
BETA = 1.8
BLUR_RADII = [11.3535, 17.9381, 5.7966, 10.8586, 5.5301, 15.9075, 12.3225, 13.4871, 6.6639, 9.5413]


def _ksize(r):
    k = int(2 * r) + 1
    return k + 1 if k % 2 == 0 else k


KSIZES = [_ksize(r) for r in BLUR_RADII]
H = W = 768
B_TOTAL, C = 16, 3
N_CORES = 8
B_LOC = B_TOTAL // N_CORES          # 2 images per core
IC = B_LOC * C                      # 6 image-channels per core
NG = IC // 2                        # 3 pairs of image-channels
P = 128
EPS = 5e-3                          # mask support threshold (error-validated)

_bf16 = ml_dtypes.bfloat16
_fp8 = ml_dtypes.float8_e4m3fn


def _conv_matrix(sigma, ksize, n=768):
    """n x n matrix Kmat with blur_1d(x) = Kmat @ x, matching the reference
    (correlation with normalized gaussian, 'reflect' padding)."""
    half = (ksize - 1) * 0.5
    xs = np.linspace(-half, half, ksize)
    pdf = np.exp(-0.5 * (xs / np.float64(sigma)) ** 2)
    k1 = (pdf / pdf.sum()).astype(np.float32).astype(np.float64)
    pad = ksize // 2
    Kmat = np.zeros((n, n), dtype=np.float64)
    idx = np.arange(n)[:, None] + np.arange(ksize)[None, :] - pad
    idx = np.abs(idx)
    idx = np.where(idx >= n, 2 * n - 2 - idx, idx)
    np.add.at(Kmat, (np.repeat(np.arange(n), ksize), idx.ravel()),
              np.tile(k1, n))
    return Kmat.astype(np.float32)


class _Drop:
    pass


def _drop_meta(positions, radius):
    """Host-side per-drop geometry + tensors (shared across cores) on the
    row-offset block grid."""
    pos = np.clip(np.asarray(positions, np.float32), -1.0, 1.0)
    rad = np.clip(np.asarray(radius, np.float32), MIN_R, MAX_R)
    s = float(np.sqrt((-np.log(EPS)) ** (1.0 / BETA)))
    s2 = s * s

    geo = []
    for j in range(NUM_DROPS):
        x0 = (pos[j, 0] + 1.0) / 2.0 * W
        y0 = (pos[j, 1] + 1.0) / 2.0 * H
        wr = rad[j]
        hr = wr * np.float32(0.8)
        p = KSIZES[j] // 2
        h0 = max(0, int(np.floor(y0 - s * hr))) & ~1
        h1 = min(H, (int(np.ceil(y0 + s * hr)) + 2) & ~1)
        w0 = max(0, int(np.floor(x0 - s * wr))) & ~1
        w1 = min(W, (int(np.ceil(x0 + s * wr)) + 2) & ~1)
        geo.append([h0, h1, w0, w1, p, float(x0), float(y0), float(wr), float(hr)])

    # pick an even grid offset so every drop spans exactly 2 blocks
    delta = None
    for dd_ in range(0, 128, 2):
        if all(((g[0] - dd_) % 128) + (g[1] - g[0]) <= 256 for g in geo):
            delta = dd_
            break
    assert delta is not None, "no 2-block grid offset exists"
    hmin = min(g[0] for g in geo)
    hmax = max(g[1] for g in geo)
    g0 = hmin - ((hmin - delta) % 128)
    NB = -((g0 - hmax) // 128)

    drops = []
    for j in range(NUM_DROPS):
        h0, h1, w0, w1, p, x0, y0, wr, hr = geo[j]
        d = _Drop()
        d.j, d.p = j, p
        d.B0 = (h0 - g0) // 128
        d.HBs = g0 + 128 * d.B0
        assert h1 - d.HBs <= 256 and d.B0 + 2 <= NB
        # cap w so Wt <= 256 (two overlapping 128-col chunks)
        wcap = 256 - 2 * p - 2
        while w1 - w0 > wcap:
            if x0 - w0 > w1 - x0:
                w0 += 2
            else:
                w1 -= 2
        d.h0, d.h1, d.w0, d.w1 = h0, h1, w0, w1
        d.span = h1 - h0
        d.Wr = w1 - w0
        d.voff = h0 - d.HBs
        wa = max(0, w0 - p) & ~1
        wb = min(W, (w1 + p + 1) & ~1)
        d.wa, d.wb = wa, wb
        d.Wt = wb - wa
        assert d.Wt <= 256 and d.span <= 256
        d.WBn = (d.Wt + P - 1) // P
        assert d.WBn == 2
        d.cstarts = [wa, wb - P]

        # pass A bands per k-block: output h' range (relative to h0)
        d.bandsA = []
        for k in range(2):
            a = max(0, d.HBs + P * k - p - h0)
            b = min(d.span, d.HBs + P * (k + 1) + p - h0)
            d.bandsA.append((a, b))

        # per h-block composite w-range [wl, wr) from the ellipse extent
        d.hbw = []
        for hb in range(2):
            ra = max(h0, d.HBs + P * hb)
            rb = min(h1, d.HBs + P * (hb + 1))
            if ra - 1 < y0 < rb:
                dh = 0.0
            else:
                dh = min(abs(ra - y0), abs(rb - 1 - y0))
            half = wr * np.sqrt(max(0.0, s2 - (dh / hr) ** 2))
            wl = max(w0, (int(np.floor(x0 - half)) - 2) & ~1)
            wr_ = min(w1, (int(np.ceil(x0 + half)) + 4) & ~1)
            wr_ = max(wr_, wl + 2)
            d.hbw.append((wl, wr_))
        # union composite window across both h-blocks (mask is zero outside
        # each block's own [wl, wr), so fused ops over the union are exact)
        d.wlu = min(wl for wl, _ in d.hbw)
        d.wru = max(wr_ for _, wr_ in d.hbw)

        # mask over [2 blocks of 128 rows] x [wa:wb], zero outside support
        rows = (d.HBs + np.arange(2 * P, dtype=np.int64)).astype(np.float32)
        dd = (rows[:, None] - y0) ** 2 / hr ** 2 + \
             (np.arange(wa, wb, dtype=np.float32)[None, :] - x0) ** 2 / wr ** 2
        m = np.clip(np.exp(-(dd.astype(np.float32) ** np.float32(BETA)) + np.float32(1e-10)), 0.0, 1.0)
        m = np.where(dd <= np.float32(s2), m, 0.0).astype(np.float32)
        mz = np.zeros_like(m)
        for hb in range(2):
            ra = max(h0, d.HBs + P * hb) - d.HBs
            rb = min(h1, d.HBs + P * (hb + 1)) - d.HBs
            wl, wr_ = d.hbw[hb]
            mz[ra:rb, wl - wa:wr_ - wa] = m[ra:rb, wl - wa:wr_ - wa]
        m1 = np.ascontiguousarray(
            mz.reshape(2, P, d.Wt).transpose(1, 0, 2)).astype(_bf16)
        d.m_np = np.ascontiguousarray(
            np.broadcast_to(m1[:, None], (P, 2, 2, d.Wt)))

        MT = _conv_matrix(BLUR_RADII[j], KSIZES[j]).T    # MT[src, dst]
        kv = np.zeros((P, 2, d.span), np.float32)
        for k in range(2):
            r0 = d.HBs + P * k
            lo = max(0, -r0)
            hi = min(P, H - r0)
            if hi > lo:
                kv[lo:hi, k, :] = MT[r0 + lo:r0 + hi, h0:h1]
        d.kv_np = np.ascontiguousarray(kv.astype(_bf16))
        kh = np.zeros((P, 2, d.Wr), np.float32)
        for wc in range(2):
            c = d.cstarts[wc]
            kh[:, wc, :] = MT[c:c + P, w0:w1]
        # the second w-chunk overlaps the first: zero duplicated rows
        dup = d.cstarts[0] + P - d.cstarts[1]
        if dup > 0:
            kh[:dup, 1, :] = 0.0
        d.kh_np = np.ascontiguousarray(kh.astype(_bf16))
        drops.append(d)
    return drops, g0, NB, hmin, hmax


def _topo_order(drops):
    """Exact dependency DAG on (block-range x w-range) slice overlap;
    emission order = stable topological levels."""
    def _dep(i, j):
        di, dj_ = drops[i], drops[j]
        if abs(di.B0 - dj_.B0) > 1:
            return False
        ri, wi = (di.wa, di.wb), (di.w0, di.w1)
        rj, wj = (dj_.wa, dj_.wb), (dj_.w0, dj_.w1)
        for (a, b) in ((wi, rj), (ri, wj), (wi, wj)):
            if max(a[0], b[0]) < min(a[1], b[1]):
                return True
        return False

    level = [0] * NUM_DROPS
    for j in range(NUM_DROPS):
        for i in range(j):
            if _dep(i, j):
                level[j] = max(level[j], level[i] + 1)
    order = sorted(range(NUM_DROPS), key=lambda j: (level[j], j))
    return order, level


def _store_pieces(drops, order, NB):
    """Per block: split [0,W) into up to 3 w-pieces, each tagged with the
    emission position of its LAST writer (-1 = never written)."""
    pieces = {}
    for blk in range(NB):
        last = np.full(W, -1, np.int64)
        for pos, dj in enumerate(order):
            d = drops[dj]
            if d.B0 <= blk <= d.B0 + 1:
                last[d.w0:d.w1] = pos
        runs = []
        ws = 0
        for x in range(1, W + 1):
            if x == W or last[x] != last[ws]:
                runs.append([ws, x, int(last[ws])])
                ws = x
        # merge small runs / cap count; merged run stores after max(pos)
        def _merge_once():
            k = min(range(len(runs)), key=lambda i: runs[i][1] - runs[i][0])
            if k == 0:
                k2 = 1
            elif k == len(runs) - 1:
                k2 = k - 1
            else:
                k2 = k - 1 if (runs[k - 1][1] - runs[k - 1][0]) < (runs[k + 1][1] - runs[k + 1][0]) else k + 1
            a, b = min(k, k2), max(k, k2)
            runs[a] = [runs[a][0], runs[b][1], max(runs[a][2], runs[b][2])]
            del runs[b]
        while len(runs) > 3 or min(r[1] - r[0] for r in runs) < 96:
            _merge_once()
        # even alignment
        for r in runs:
            r[0] &= ~1
        for i in range(len(runs) - 1):
            runs[i][1] = runs[i + 1][0]
        runs[-1][1] = W
        pieces[blk] = [(r[0], r[1], r[2]) for r in runs]
    return pieces


class _Balancer:
    """Greedy static load-balancer across Vector/Scalar/GpSimd with
    HW-measured per-op costs (ns): V sbuf-bf16 TT ~0.62/elem (2x mode),
    V psum-touching 1.1/elem, S copy 1.15/elem, G TT 2.0/elem.
    S (Activation) can only copy; G cannot touch PSUM."""

    def __init__(self, nc):
        self.nc = nc
        self.load = {'V': 0.0, 'S': 0.0, 'G': 0.0}

    def _pick(self, costs):
        eng, c = min(costs, key=lambda ec: self.load[ec[0]] + ec[1])
        self.load[eng] += c
        return eng

    def tt(self, op, out, a, b, fd):
        costs = [('V', fd * 0.62 + 150), ('G', fd * 2.0 + 220)]
        eng = self._pick(costs)
        e = self.nc.vector if eng == 'V' else self.nc.gpsimd
        getattr(e, 'tensor_' + op)(out, a, b)

    def copy(self, out, src, fd):
        # PSUM f32 -> SBUF (V at 1x psum rate, S activation copy)
        eng = self._pick([('V', fd * 1.1 + 200), ('S', fd * 1.15 + 200)])
        if eng == 'V':
            self.nc.vector.tensor_copy(out, src)
        else:
            self.nc.scalar.copy(out=out, in_=src)

    def bsh_mul(self, psb_sl, bshp, m_sl, t2_sl, fd, shape, dt):
        """t2 = m * psb, either via {S|V} psum-copy + {V|G} bf16 mul, or
        V direct mul from PSUM."""
        cV, cS = fd * 1.1 + 200, fd * 1.15 + 200
        mV, mG = fd * 0.62 + 150, fd * 2.0 + 220
        dV = fd * 1.1 + 200
        best, opt = None, None
        for tag, deltas in [('SV', (('S', cS), ('V', mV))),
                            ('SG', (('S', cS), ('G', mG))),
                            ('VG', (('V', cV), ('G', mG))),
                            ('D', (('V', dV),))]:
            tmp = dict(self.load)
            for e, c in deltas:
                tmp[e] += c
            key = (max(tmp.values()), sum(tmp.values()))
            if best is None or key < best:
                best, opt = key, (tag, deltas)
        tag, deltas = opt
        for e, c in deltas:
            self.load[e] += c
        if tag == 'D':
            self.nc.vector.tensor_mul(t2_sl, m_sl, psb_sl)
        else:
            bsh = bshp.tile(shape, dt, tag="Bs")
            bsh_sl = bsh[:, :, :, 0:psb_sl.shape[-1]]
            if tag[0] == 'S':
                self.nc.scalar.copy(out=bsh_sl, in_=psb_sl)
            else:
                self.nc.vector.tensor_copy(bsh_sl, psb_sl)
            e = self.nc.vector if tag[1] == 'V' else self.nc.gpsimd
            e.tensor_mul(t2_sl, m_sl, bsh_sl)


def _build_program(drops, g0, NB, hmin, hmax, order, lvl, pieces):
    from contextlib import ExitStack
    from concourse import bacc, tile, mybir

    f32 = mybir.dt.float32
    bf16 = mybir.dt.bfloat16
    fp8 = mybir.dt.float8e4

    nc = bacc.Bacc("TRN2", target_bir_lowering=False, debug=False,
                   num_devices=N_CORES)

    # input params: per (pair, blk), always 128 partitions (host zero-pads
    # rows outside [hmin, hmax))
    imgs_d = [[nc.declare_dram_parameter(f"i{g}b{blk}", [P, 2, W], bf16, False)
               for blk in range(NB)] for g in range(NG)]
    # output params: per (pair, blk, piece), partial partitions on edge blocks
    pu = {blk: (max(0, hmin - (g0 + 128 * blk)),
                min(P, hmax - (g0 + 128 * blk))) for blk in range(NB)}
    outs_d = {}
    for g in range(NG):
        for blk in range(NB):
            p0, p1 = pu[blk]
            for pi, (ws, we, _pos) in enumerate(pieces[blk]):
                outs_d[(g, blk, pi)] = nc.declare_dram_parameter(
                    f"o{g}b{blk}p{pi}", [p1 - p0, 2, we - ws], bf16, True)

    # drop params batched into 3 chunks by emission order
    chunks = [order[0:2], order[2:4], order[4:]]
    kvoffs, khoffs = {}, {}
    kvlen = [0] * len(chunks)
    khlen = [0] * len(chunks)
    for ci, ch in enumerate(chunks):
        for dj in ch:
            d = drops[dj]
            kvoffs[dj] = (ci, kvlen[ci])
            khoffs[dj] = (ci, khlen[ci])
            kvlen[ci] += 2 * d.span
            khlen[ci] += 2 * d.Wr
    WMAX = 256
    pchunks = []
    for ci, ch in enumerate(chunks):
        pchunks.append((
            nc.declare_dram_parameter(f"mc{ci}", [P, len(ch), 2, 2, WMAX], bf16, False),
            nc.declare_dram_parameter(f"kvc{ci}", [P, kvlen[ci]], bf16, False),
            nc.declare_dram_parameter(f"khc{ci}", [P, khlen[ci]], bf16, False)))

    bal = _Balancer(nc)

    with tile.TileContext(nc) as tc, ExitStack() as ctx:
        outp = ctx.enter_context(tc.tile_pool(name="out_state", bufs=1))
        out_s = outp.tile([P, IC, NB, W], bf16, name="state", tag="state")
        dp = ctx.enter_context(tc.tile_pool(name="dropin", bufs=1))
        omp = ctx.enter_context(tc.tile_pool(name="omq", bufs=8))
        vtp = ctx.enter_context(tc.tile_pool(name="vts", bufs=8))
        bshp = ctx.enter_context(tc.tile_pool(name="bsh", bufs=8))
        ppa = ctx.enter_context(tc.tile_pool(name="psa", bufs=2, space="PSUM"))
        ppb = ctx.enter_context(tc.tile_pool(name="psb", bufs=2, space="PSUM"))

        # ---- PE warm-up: matmuls on a zeroed tile span the load window
        wt = dp.tile([P, 512], bf16, tag="warm")
        nc.gpsimd.memset(wt[:], 0)
        warm = ppa.tile([P, 2, 2, 256], f32, tag="psa")
        for i in range(46):
            nc.tensor.matmul(warm[:, 0, 0, 0:256], lhsT=wt[:, 0:P],
                             rhs=wt[:, 0:256], start=True, stop=True)
        # pre-zero the vt ring so pass-B stationaries never read NaN garbage
        for i in range(4):
            v0 = vtp.tile([P, 2, 2, 256], bf16, tag="vt", bufs=4)
            (nc.vector if i % 2 else nc.gpsimd).memset(v0[:], 0)

        # ---- loads: params chunk0 on scalar; imgs pair-major on sync so
        # each drop chain starts as its blocks arrive; later chunks follow
        ptiles = []
        for ci, ch in enumerate(chunks):
            ptiles.append((
                dp.tile([P, len(ch), 2, 2, WMAX], bf16, tag=f"mc{ci}", name=f"mc{ci}"),
                dp.tile([P, kvlen[ci]], bf16, tag=f"kvc{ci}", name=f"kvc{ci}"),
                dp.tile([P, khlen[ci]], bf16, tag=f"khc{ci}", name=f"khc{ci}")))
        for t, pd in zip(ptiles[0], pchunks[0]):
            nc.scalar.dma_start(out=t[:], in_=pd.ap()[:])
        for g in range(NG):
            for blk in range(NB):
                nc.sync.dma_start(out=out_s[:, 2 * g:2 * g + 2, blk, :],
                                  in_=imgs_d[g][blk].ap()[:])
        for ci in (1, 2):
            for t, pd in zip(ptiles[ci], pchunks[ci]):
                nc.scalar.dma_start(out=t[:], in_=pd.ap()[:])

        # position of each drop in emission order, for store scheduling
        pos_of = {dj: pos for pos, dj in enumerate(order)}
        store_after = {}
        for blk in range(NB):
            for pi, (ws, we, pos) in enumerate(pieces[blk]):
                store_after.setdefault(max(pos, 0), []).append((blk, pi, ws, we))

        # ---- drops: software-pipelined at (drop, pair) granularity so no
        # engine FIFO blocks at its head and PSUM rings (2 bufs each) are
        # recycled only after their reader is emitted.
        #   iteration t: comp(u[t-2]) -> evict+q+passB(u[t-1]) -> om+passA(u[t])
        waves = {}
        for dj in order:
            waves.setdefault(lvl[dj], []).append(dj)

        class _U:
            pass

        def stage1(dj, g):
            u = _U()
            d = drops[dj]
            ci = next(ii for ii, ch in enumerate(chunks) if dj in ch)
            u.d, u.g, u.dj = d, g, dj
            u.i = chunks[ci].index(dj)
            u.mt, u.kvt, u.kht = ptiles[ci]
            _, u.kvo = kvoffs[dj]
            _, u.kho = khoffs[dj]
            sl = out_s[:, 2 * g:2 * g + 2, d.B0:d.B0 + 2, d.wa:d.wb]
            u.om = omp.tile([P, 2, 2, 256], bf16, tag="om", bufs=4)
            bal.tt('mul', u.om[:, :, :, 0:d.Wt],
                   u.mt[:, u.i, 0:2, 0:2, 0:d.Wt], sl, 4 * d.Wt)
            u.psa = ppa.tile([P, 2, 2, 256], f32, tag="psa", bufs=2)
            for wc in range(2):
                coff = d.cstarts[wc] - d.wa
                for jj in range(2):
                    for k in range(2):
                        a, b = d.bandsA[k]
                        nc.tensor.matmul(
                            u.psa[:, jj, wc, a:b],
                            lhsT=u.om[:, jj, k, coff:coff + P],
                            rhs=u.kvt[:, u.kvo + k * d.span + a:u.kvo + k * d.span + b],
                            start=(k == 0), stop=(k == 1))
            return u

        def stage2(u):
            d, g = u.d, u.g
            au, bu = d.wlu - d.w0, d.wru - d.w0
            u.vt = vtp.tile([P, 2, 2, 256], bf16, tag="vt", bufs=4)
            bal.copy(u.vt[:, :, :, d.voff:d.voff + d.span],
                     u.psa[:, :, :, 0:d.span], 4 * d.span)
            slq = out_s[:, 2 * g:2 * g + 2, d.B0:d.B0 + 2, d.w0:d.w1]
            u.q = omp.tile([P, 2, 2, 256], bf16, tag="q", bufs=4)
            bal.tt('sub', u.q[:, :, :, 0:d.Wr], slq,
                   u.om[:, :, :, d.w0 - d.wa:d.w0 - d.wa + d.Wr], 4 * d.Wr)
            u.psb = ppb.tile([P, 2, 2, 256], f32, tag="psb", bufs=2)
            for jj in range(2):
                for hb in range(2):
                    for wc in range(2):
                        nc.tensor.matmul(
                            u.psb[:, jj, hb, au:bu],
                            lhsT=u.vt[:, jj, wc, hb * P:(hb + 1) * P],
                            rhs=u.kht[:, u.kho + wc * d.Wr + au:u.kho + wc * d.Wr + bu],
                            start=(wc == 0), stop=(wc == 1))

        def stage3(u):
            d, g = u.d, u.g
            au, bu = d.wlu - d.w0, d.wru - d.w0
            widu = bu - au
            t2 = bshp.tile([P, 2, 2, 256], bf16, tag="t2", bufs=4)
            bal.bsh_mul(u.psb[:, :, :, au:bu], bshp,
                        u.mt[:, u.i, 0:2, 0:2, d.wlu - d.wa:d.wru - d.wa],
                        t2[:, :, :, 0:widu], 4 * widu, [P, 2, 2, 256], bf16)
            osl = out_s[:, 2 * g:2 * g + 2, d.B0:d.B0 + 2, d.wlu:d.wru]
            bal.tt('add', osl, u.q[:, :, :, au:bu], t2[:, :, :, 0:widu],
                   4 * widu)
            if g == NG - 1:
                for (blk, pi, ws, we) in store_after.get(pos_of[u.dj], []):
                    p0, p1 = pu[blk]
                    for gg in range(NG):
                        nc.sync.dma_start(
                            out=outs_d[(gg, blk, pi)].ap()[:],
                            in_=out_s[p0:p1, 2 * gg:2 * gg + 2, blk, ws:we])

        for lv in sorted(waves):
            units = [(dj, g) for dj in waves[lv] for g in range(NG)]
            ring = []
            for t in range(len(units) + 2):
                if t >= 2:
                    stage3(ring[t - 2])
                if t >= 1 and t - 1 < len(units):
                    stage2(ring[t - 1])
                if t < len(units):
                    ring.append(stage1(*units[t]))
    nc.compile()
    print("balancer loads (us):",
          {k: round(v / 1000, 1) for k, v in bal.load.items()})
    return nc


_CACHE = {}


def _get_program(positions, radius):
    key = (np.asarray(positions, np.float32).tobytes(),
           np.asarray(radius, np.float32).tobytes())
    if key not in _CACHE:
        drops, g0, NB, hmin, hmax = _drop_meta(positions, radius)
        order, level = _topo_order(drops)
        print("emission order:", order, "levels:", level)
        pieces = _store_pieces(drops, order, NB)
        nc = _build_program(drops, g0, NB, hmin, hmax, order, level, pieces)
        _CACHE[key] = (nc, drops, g0, NB, hmin, hmax, order, pieces)
    return _CACHE[key]


def kernel(img, positions, radius, _want_trace=False, **_kw):
    from concourse.bass_utils import run_bass_kernel_spmd
    img = np.asarray(img, np.float32)
    assert img.shape == (B_TOTAL, C, H, W)
    nc, drops, g0, NB, hmin, hmax, order, pieces = _get_program(positions, radius)

    # pack rows [g0, g0+NB*128) to [p, pair(2), w] per (core, pair, blk), bf16,
    # zero-padded outside [hmin, hmax)
    rows_lo, rows_hi = hmin, hmax
    imgb = np.zeros((N_CORES, IC, NB * P, W), _bf16)
    src = img.reshape(N_CORES, IC, H, W)
    imgb[:, :, rows_lo - g0:rows_hi - g0, :] = src[:, :, rows_lo:rows_hi, :].astype(_bf16)
    packed = np.ascontiguousarray(
        imgb.reshape(N_CORES, IC, NB, P, W).transpose(0, 3, 1, 2, 4))

    chunks = [order[0:2], order[2:4], order[4:]]
    WMAX = 256
    base = {}
    for ci, ch in enumerate(chunks):
        mc = np.zeros((P, len(ch), 2, 2, WMAX), _bf16)
        for i, dj in enumerate(ch):
            d = drops[dj]
            mc[:, i, :, :, 0:d.Wt] = d.m_np
        base[f"mc{ci}"] = mc
        base[f"kvc{ci}"] = np.ascontiguousarray(np.concatenate(
            [drops[dj].kv_np.reshape(P, -1) for dj in ch], axis=1))
        base[f"khc{ci}"] = np.ascontiguousarray(np.concatenate(
            [drops[dj].kh_np.reshape(P, -1) for dj in ch], axis=1))
    in_maps = []
    for i in range(N_CORES):
        mp = dict(base)
        for g in range(NG):
            for blk in range(NB):
                mp[f"i{g}b{blk}"] = np.ascontiguousarray(
                    packed[i][:, 2 * g:2 * g + 2, blk, :])
        in_maps.append(mp)
    res = run_bass_kernel_spmd(nc, in_maps, core_ids=list(range(N_CORES)),
                               trace=_want_trace)
    out = img.copy()
    pu = {blk: (max(0, hmin - (g0 + 128 * blk)),
                min(P, hmax - (g0 + 128 * blk))) for blk in range(NB)}
    for i in range(N_CORES):
        oc = out.reshape(N_CORES, IC, H, W)
        for g in range(NG):
            for blk in range(NB):
                p0, p1 = pu[blk]
                r0 = g0 + 128 * blk + p0
                for pi, (ws, we, _pos) in enumerate(pieces[blk]):
                    blkres = res.results[i][f"o{g}b{blk}p{pi}"]
                    # [Pu, 2, wlen] -> rows r0..r0+Pu
                    oc[i, 2 * g:2 * g + 2, r0:r0 + (p1 - p0), ws:we] = \
                        blkres.transpose(1, 0, 2).astype(np.float32)
    if _want_trace:
        return out, res
    return out


# revision 22
# speedup vs baseline: 1.0831x; 1.0323x over previous
"""Trainium2 Bass kernel for nn_Condensation: 10 sequential masked-Gaussian-blur
composites over a [16,3,768,768] image, data-parallel over 8 NeuronCores.

v4 strategy (per core, 2 images = 6 image-channels):
  - Row-offset block grid (delta chosen so EVERY drop's mask support fits in
    exactly 2 h-blocks of 128 rows). Cuts elementwise/mask/matmul work ~25%
    vs a 0-based grid (where 5 drops straddled 3 blocks) and removes the
    false inter-drop dependencies block padding created.
  - Drops emitted in exact topological order of their true spatial overlap
    DAG (non-overlapping drops commute): 3 levels of 4/4/2 drops in flight
    instead of 5 waves of 2 -> much better engine ILP.
  - State resident in SBUF as bf16 [128, 6, NB, 768]; only the 416 rows any
    drop touches are loaded/stored (partial-partition edge blocks, zero-
    padded loads); host copies untouched rows.
  - Separable blur as two banded-matmul passes on TensorE (bf16, f32 PSUM),
    support-clipped bands; q-trick composite (q = out - om) with per-op
    greedy balancing across Vector/Scalar/GpSimd using trace-calibrated
    costs (V bf16 2x-mode vs PSUM 1x, S copy-only, G slow).
  - Stores split per (block, w-piece) keyed to each piece's LAST writer in
    emission order, so most output DMA drains long before the final drop.
"""
import numpy as np
import ml_dtypes

NUM_DROPS = 10
MIN_R, MAX_R = 60.0, 80.0
BETA = 1.8
BLUR_RADII = [11.3535, 17.9381, 5.7966, 10.8586, 5.5301, 15.9075, 12.3225, 13.4871, 6.6639, 9.5413]


def _ksize(r):
    k = int(2 * r) + 1
    return k + 1 if k % 2 == 0 else k


KSIZES = [_ksize(r) for r in BLUR_RADII]
H = W = 768
B_TOTAL, C = 16, 3
N_CORES = 8
B_LOC = B_TOTAL // N_CORES          # 2 images per core
IC = B_LOC * C                      # 6 image-channels per core
NG = IC // 2                        # 3 pairs of image-channels
P = 128
EPS = 5e-3                          # mask support threshold (error-validated)

_bf16 = ml_dtypes.bfloat16
_fp8 = ml_dtypes.float8_e4m3fn


def _conv_matrix(sigma, ksize, n=768):
    """n x n matrix Kmat with blur_1d(x) = Kmat @ x, matching the reference
    (correlation with normalized gaussian, 'reflect' padding)."""
    half = (ksize - 1) * 0.5
    xs = np.linspace(-half, half, ksize)
    pdf = np.exp(-0.5 * (xs / np.float64(sigma)) ** 2)
    k1 = (pdf / pdf.sum()).astype(np.float32).astype(np.float64)
    pad = ksize // 2
    Kmat = np.zeros((n, n), dtype=np.float64)
    idx = np.arange(n)[:, None] + np.arange(ksize)[None, :] - pad
    idx = np.abs(idx)
    idx = np.where(idx >= n, 2 * n - 2 - idx, idx)
    np.add.at(Kmat, (np.repeat(np.arange(n), ksize), idx.ravel()),
              np.tile(k1, n))
    return Kmat.astype(np.float32)


class _Drop:
    pass


def _drop_meta(positions, radius):
    """Host-side per-drop geometry + tensors (shared across cores) on the
    row-offset block grid."""
    pos = np.clip(np.asarray(positions, np.float32), -1.0, 1.0)
    rad = np.clip(np.asarray(radius, np.float32), MIN_R, MAX_R)
    s = float(np.sqrt((-np.log(EPS)) ** (1.0 / BETA)))
    s2 = s * s

    geo = []
    for j in range(NUM_DROPS):
        x0 = (pos[j, 0] + 1.0) / 2.0 * W
        y0 = (pos[j, 1] + 1.0) / 2.0 * H
        wr = rad[j]
        hr = wr * np.float32(0.8)
        p = KSIZES[j] // 2
        h0 = max(0, int(np.floor(y0 - s * hr))) & ~1
        h1 = min(H, (int(np.ceil(y0 + s * hr)) + 2) & ~1)
        w0 = max(0, int(np.floor(x0 - s * wr))) & ~1
        w1 = min(W, (int(np.ceil(x0 + s * wr)) + 2) & ~1)
        geo.append([h0, h1, w0, w1, p, float(x0), float(y0), float(wr), float(hr)])

    # pick an even grid offset so every drop spans exactly 2 blocks
    delta = None
    for dd_ in range(0, 128, 2):
        if all(((g[0] - dd_) % 128) + (g[1] - g[0]) <= 256 for g in geo):
            delta = dd_
            break
    assert delta is not None, "no 2-block grid offset exists"
    hmin = min(g[0] for g in geo)
    hmax = max(g[1] for g in geo)
    g0 = hmin - ((hmin - delta) % 128)
    NB = -((g0 - hmax) // 128)

    drops = []
    for j in range(NUM_DROPS):
        h0, h1, w0, w1, p, x0, y0, wr, hr = geo[j]
        d = _Drop()
        d.j, d.p = j, p
        d.B0 = (h0 - g0) // 128
        d.HBs = g0 + 128 * d.B0
        assert h1 - d.HBs <= 256 and d.B0 + 2 <= NB
        # cap w so Wt <= 256 (two overlapping 128-col chunks)
        wcap = 256 - 2 * p - 2
        while w1 - w0 > wcap:
            if x0 - w0 > w1 - x0:
                w0 += 2
            else:
                w1 -= 2
        d.h0, d.h1, d.w0, d.w1 = h0, h1, w0, w1
        d.span = h1 - h0
        d.Wr = w1 - w0
        d.voff = h0 - d.HBs
        wa = max(0, w0 - p) & ~1
        wb = min(W, (w1 + p + 1) & ~1)
        d.wa, d.wb = wa, wb
        d.Wt = wb - wa
        assert d.Wt <= 256 and d.span <= 256
        d.WBn = (d.Wt + P - 1) // P
        assert d.WBn == 2
        d.cstarts = [wa, wb - P]

        # pass A bands per k-block: output h' range (relative to h0)
        d.bandsA = []
        for k in range(2):
            a = max(0, d.HBs + P * k - p - h0)
            b = min(d.span, d.HBs + P * (k + 1) + p - h0)
            d.bandsA.append((a, b))

        # per h-block composite w-range [wl, wr) from the ellipse extent
        d.hbw = []
        for hb in range(2):
            ra = max(h0, d.HBs + P * hb)
            rb = min(h1, d.HBs + P * (hb + 1))
            if ra - 1 < y0 < rb:
                dh = 0.0
            else:
                dh = min(abs(ra - y0), abs(rb - 1 - y0))
            half = wr * np.sqrt(max(0.0, s2 - (dh / hr) ** 2))
            wl = max(w0, (int(np.floor(x0 - half)) - 2) & ~1)
            wr_ = min(w1, (int(np.ceil(x0 + half)) + 4) & ~1)
            wr_ = max(wr_, wl + 2)
            d.hbw.append((wl, wr_))
        # union composite window across both h-blocks (mask is zero outside
        # each block's own [wl, wr), so fused ops over the union are exact)
        d.wlu = min(wl for wl, _ in d.hbw)
        d.wru = max(wr_ for _, wr_ in d.hbw)

        # mask over [2 blocks of 128 rows] x [wa:wb], zero outside support
        rows = (d.HBs + np.arange(2 * P, dtype=np.int64)).astype(np.float32)
        dd = (rows[:, None] - y0) ** 2 / hr ** 2 + \
             (np.arange(wa, wb, dtype=np.float32)[None, :] - x0) ** 2 / wr ** 2
        m = np.clip(np.exp(-(dd.astype(np.float32) ** np.float32(BETA)) + np.float32(1e-10)), 0.0, 1.0)
        m = np.where(dd <= np.float32(s2), m, 0.0).astype(np.float32)
        mz = np.zeros_like(m)
        for hb in range(2):
            ra = max(h0, d.HBs + P * hb) - d.HBs
            rb = min(h1, d.HBs + P * (hb + 1)) - d.HBs
            wl, wr_ = d.hbw[hb]
            mz[ra:rb, wl - wa:wr_ - wa] = m[ra:rb, wl - wa:wr_ - wa]
        m1 = np.ascontiguousarray(
            mz.reshape(2, P, d.Wt).transpose(1, 0, 2)).astype(_bf16)
        d.m_np = np.ascontiguousarray(
            np.broadcast_to(m1[:, None], (P, 2, 2, d.Wt)))

        MT = _conv_matrix(BLUR_RADII[j], KSIZES[j]).T    # MT[src, dst]
        kv = np.zeros((P, 2, d.span), np.float32)
        for k in range(2):
            r0 = d.HBs + P * k
            lo = max(0, -r0)
            hi = min(P, H - r0)
            if hi > lo:
                kv[lo:hi, k, :] = MT[r0 + lo:r0 + hi, h0:h1]
        d.kv_np = np.ascontiguousarray(kv.astype(_bf16))
        kh = np.zeros((P, 2, d.Wr), np.float32)
        for wc in range(2):
            c = d.cstarts[wc]
            kh[:, wc, :] = MT[c:c + P, w0:w1]
        # the second w-chunk overlaps the first: zero duplicated rows
        dup = d.cstarts[0] + P - d.cstarts[1]
        if dup > 0:
            kh[:dup, 1, :] = 0.0
        d.kh_np = np.ascontiguousarray(kh.astype(_bf16))
        drops.append(d)
    return drops, g0, NB, hmin, hmax


def _topo_order(drops):
    """Exact dependency DAG on (block-range x w-range) slice overlap;
    emission order = stable topological levels."""
    def _dep(i, j):
        di, dj_ = drops[i], drops[j]
        if abs(di.B0 - dj_.B0) > 1:
            return False
        ri, wi = (di.wa, di.wb), (di.w0, di.w1)
        rj, wj = (dj_.wa, dj_.wb), (dj_.w0, dj_.w1)
        for (a, b) in ((wi, rj), (ri, wj), (wi, wj)):
            if max(a[0], b[0]) < min(a[1], b[1]):
                return True
        return False

    level = [0] * NUM_DROPS
    for j in range(NUM_DROPS):
        for i in range(j):
            if _dep(i, j):
                level[j] = max(level[j], level[i] + 1)
    order = sorted(range(NUM_DROPS), key=lambda j: (level[j], j))
    return order, level


def _store_pieces(drops, order, NB):
    """Per block: split [0,W) into up to 3 w-pieces, each tagged with the
    emission position of its LAST writer (-1 = never written)."""
    pieces = {}
    for blk in range(NB):
        last = np.full(W, -1, np.int64)
        for pos, dj in enumerate(order):
            d = drops[dj]
            if d.B0 <= blk <= d.B0 + 1:
                last[d.w0:d.w1] = pos
        runs = []
        ws = 0
        for x in range(1, W + 1):
            if x == W or last[x] != last[ws]:
                runs.append([ws, x, int(last[ws])])
                ws = x
        # merge small runs / cap count; merged run stores after max(pos)
        def _merge_once():
            k = min(range(len(runs)), key=lambda i: runs[i][1] - runs[i][0])
            if k == 0:
                k2 = 1
            elif k == len(runs) - 1:
                k2 = k - 1
            else:
                k2 = k - 1 if (runs[k - 1][1] - runs[k - 1][0]) < (runs[k + 1][1] - runs[k + 1][0]) else k + 1
            a, b = min(k, k2), max(k, k2)
            runs[a] = [runs[a][0], runs[b][1], max(runs[a][2], runs[b][2])]
            del runs[b]
        while len(runs) > 3 or min(r[1] - r[0] for r in runs) < 96:
            _merge_once()
        # even alignment
        for r in runs:
            r[0] &= ~1
        for i in range(len(runs) - 1):
            runs[i][1] = runs[i + 1][0]
        runs[-1][1] = W
        pieces[blk] = [(r[0], r[1], r[2]) for r in runs]
    return pieces


class _Balancer:
    """Greedy static load-balancer across Vector/Scalar/GpSimd with
    HW-measured per-op costs (ns): V sbuf-bf16 TT ~0.62/elem (2x mode),
    V psum-touching 1.1/elem, S copy 1.15/elem, G TT 2.0/elem.
    S (Activation) can only copy; G cannot touch PSUM."""

    def __init__(self, nc):
        self.nc = nc
        self.load = {'V': 0.0, 'S': 0.0, 'G': 0.0}

    def _pick(self, costs):
        eng, c = min(costs, key=lambda ec: self.load[ec[0]] + ec[1])
        self.load[eng] += c
        return eng

    def tt(self, op, out, a, b, fd):
        costs = [('V', fd * 0.62 + 150), ('G', fd * 2.0 + 220)]
        eng = self._pick(costs)
        e = self.nc.vector if eng == 'V' else self.nc.gpsimd
        getattr(e, 'tensor_' + op)(out, a, b)

    def copy(self, out, src, fd):
        # PSUM f32 -> SBUF (V at 1x psum rate, S activation copy)
        eng = self._pick([('V', fd * 1.1 + 200), ('S', fd * 1.15 + 200)])
        if eng == 'V':
            self.nc.vector.tensor_copy(out, src)
        else:
            self.nc.scalar.copy(out=out, in_=src)

    def bsh_mul(self, psb_sl, bshp, m_sl, t2_sl, fd, shape, dt):
        """t2 = m * psb, either via {S|V} psum-copy + {V|G} bf16 mul, or
        V direct mul from PSUM."""
        cV, cS = fd * 1.1 + 200, fd * 1.15 + 200
        mV, mG = fd * 0.62 + 150, fd * 2.0 + 220
        dV = fd * 1.1 + 200
        best, opt = None, None
        for tag, deltas in [('SV', (('S', cS), ('V', mV))),
                            ('SG', (('S', cS), ('G', mG))),
                            ('VG', (('V', cV), ('G', mG))),
                            ('D', (('V', dV),))]:
            tmp = dict(self.load)
            for e, c in deltas:
                tmp[e] += c
            key = (max(tmp.values()), sum(tmp.values()))
            if best is None or key < best:
                best, opt = key, (tag, deltas)
        tag, deltas = opt
        for e, c in deltas:
            self.load[e] += c
        if tag == 'D':
            self.nc.vector.tensor_mul(t2_sl, m_sl, psb_sl)
        else:
            bsh = bshp.tile(shape, dt, tag="Bs")
            bsh_sl = bsh[:, :, :, 0:psb_sl.shape[-1]]
            if tag[0] == 'S':
                self.nc.scalar.copy(out=bsh_sl, in_=psb_sl)
            else:
                self.nc.vector.tensor_copy(bsh_sl, psb_sl)
            e = self.nc.vector if tag[1] == 'V' else self.nc.gpsimd
            e.tensor_mul(t2_sl, m_sl, bsh_sl)


def _build_program(drops, g0, NB, hmin, hmax, order, lvl, pieces):
    from contextlib import ExitStack
    from concourse import bacc, tile, mybir

    f32 = mybir.dt.float32
    bf16 = mybir.dt.bfloat16
    fp8 = mybir.dt.float8e4

    nc = bacc.Bacc("TRN2", target_bir_lowering=False, debug=False,
                   num_devices=N_CORES)

    # input params: per (pair, blk), always 128 partitions (host zero-pads
    # rows outside [hmin, hmax))
    imgs_d = [[nc.declare_dram_parameter(f"i{g}b{blk}", [P, 2, W], bf16, False)
               for blk in range(NB)] for g in range(NG)]
    # output params: per (pair, blk, piece), partial partitions on edge blocks
    pu = {blk: (max(0, hmin - (g0 + 128 * blk)),
                min(P, hmax - (g0 + 128 * blk))) for blk in range(NB)}
    outs_d = {}
    for g in range(NG):
        for blk in range(NB):
            p0, p1 = pu[blk]
            for pi, (ws, we, _pos) in enumerate(pieces[blk]):
                outs_d[(g, blk, pi)] = nc.declare_dram_parameter(
                    f"o{g}b{blk}p{pi}", [p1 - p0, 2, we - ws], bf16, True)

    # drop params batched into 3 chunks by emission order
    chunks = [order[0:2], order[2:4], order[4:]]
    kvoffs, khoffs = {}, {}
    kvlen = [0] * len(chunks)
    khlen = [0] * len(chunks)
    for ci, ch in enumerate(chunks):
        for dj in ch:
            d = drops[dj]
            kvoffs[dj] = (ci, kvlen[ci])
            khoffs[dj] = (ci, khlen[ci])
            kvlen[ci] += 2 * d.span
            khlen[ci] += 2 * d.Wr
    WMAX = 256
    pchunks = []
    for ci, ch in enumerate(chunks):
        pchunks.append((
            nc.declare_dram_parameter(f"mc{ci}", [P, len(ch), 2, 2, WMAX], bf16, False),
            nc.declare_dram_parameter(f"kvc{ci}", [P, kvlen[ci]], bf16, False),
            nc.declare_dram_parameter(f"khc{ci}", [P, khlen[ci]], bf16, False)))

    bal = _Balancer(nc)

    with tile.TileContext(nc) as tc, ExitStack() as ctx:
        outp = ctx.enter_context(tc.tile_pool(name="out_state", bufs=1))
        out_s = outp.tile([P, IC, NB, W], bf16, name="state", tag="state")
        dp = ctx.enter_context(tc.tile_pool(name="dropin", bufs=1))
        omp = ctx.enter_context(tc.tile_pool(name="omq", bufs=8))
        vtp = ctx.enter_context(tc.tile_pool(name="vts", bufs=8))
        bshp = ctx.enter_context(tc.tile_pool(name="bsh", bufs=8))
        ppa = ctx.enter_context(tc.tile_pool(name="psa", bufs=2, space="PSUM"))
        ppb = ctx.enter_context(tc.tile_pool(name="psb", bufs=2, space="PSUM"))

        # ---- PE warm-up: matmuls on a zeroed tile span the load window
        wt = dp.tile([P, 512], bf16, tag="warm")
        nc.gpsimd.memset(wt[:], 0)
        warm = ppa.tile([P, 2, 2, 256], f32, tag="psa")
        for i in range(22):
            nc.tensor.matmul(warm[:, 0, 0, 0:256], lhsT=wt[:, 0:P],
                             rhs=wt[:, 0:256], start=True, stop=True)
        # pre-zero the vt ring so pass-B stationaries never read NaN garbage
        for i in range(4):
            v0 = vtp.tile([P, 2, 2, 256], bf16, tag="vt", bufs=4)
            (nc.vector if i % 2 else nc.gpsimd).memset(v0[:], 0)

        # ---- loads: params chunk0 on scalar; imgs pair-major on sync so
        # each drop chain starts as its blocks arrive; later chunks follow
        ptiles = []
        for ci, ch in enumerate(chunks):
            ptiles.append((
                dp.tile([P, len(ch), 2, 2, WMAX], bf16, tag=f"mc{ci}", name=f"mc{ci}"),
                dp.tile([P, kvlen[ci]], bf16, tag=f"kvc{ci}", name=f"kvc{ci}"),
                dp.tile([P, khlen[ci]], bf16, tag=f"khc{ci}", name=f"khc{ci}")))
        for t, pd in zip(ptiles[0], pchunks[0]):
            nc.scalar.dma_start(out=t[:], in_=pd.ap()[:])
        for g in range(NG):
            for blk in range(NB):
                nc.sync.dma_start(out=out_s[:, 2 * g:2 * g + 2, blk, :],
                                  in_=imgs_d[g][blk].ap()[:])
        for ci in (1, 2):
            for t, pd in zip(ptiles[ci], pchunks[ci]):
                nc.scalar.dma_start(out=t[:], in_=pd.ap()[:])

        # position of each drop in emission order, for store scheduling
        pos_of = {dj: pos for pos, dj in enumerate(order)}
        store_after = {}
        for blk in range(NB):
            for pi, (ws, we, pos) in enumerate(pieces[blk]):
                store_after.setdefault(max(pos, 0), []).append((blk, pi, ws, we))

        # ---- drops: software-pipelined at (drop, pair) granularity so no
        # engine FIFO blocks at its head and PSUM rings (2 bufs each) are
        # recycled only after their reader is emitted.
        #   iteration t: comp(u[t-2]) -> evict+q+passB(u[t-1]) -> om+passA(u[t])
        waves = {}
        for dj in order:
            waves.setdefault(lvl[dj], []).append(dj)

        class _U:
            pass

        def stage1(dj, g):
            u = _U()
            d = drops[dj]
            ci = next(ii for ii, ch in enumerate(chunks) if dj in ch)
            u.d, u.g, u.dj = d, g, dj
            u.i = chunks[ci].index(dj)
            u.mt, u.kvt, u.kht = ptiles[ci]
            _, u.kvo = kvoffs[dj]
            _, u.kho = khoffs[dj]
            sl = out_s[:, 2 * g:2 * g + 2, d.B0:d.B0 + 2, d.wa:d.wb]
            u.om = omp.tile([P, 2, 2, 256], bf16, tag="om", bufs=4)
            bal.tt('mul', u.om[:, :, :, 0:d.Wt],
                   u.mt[:, u.i, 0:2, 0:2, 0:d.Wt], sl, 4 * d.Wt)
            u.psa = ppa.tile([P, 2, 2, 256], f32, tag="psa", bufs=2)
            for wc in range(2):
                coff = d.cstarts[wc] - d.wa
                for jj in range(2):
                    for k in range(2):
                        a, b = d.bandsA[k]
                        nc.tensor.matmul(
                            u.psa[:, jj, wc, a:b],
                            lhsT=u.om[:, jj, k, coff:coff + P],
                            rhs=u.kvt[:, u.kvo + k * d.span + a:u.kvo + k * d.span + b],
                            start=(k == 0), stop=(k == 1))
            return u

        def stage2(u):
            d, g = u.d, u.g
            au, bu = d.wlu - d.w0, d.wru - d.w0
            u.vt = vtp.tile([P, 2, 2, 256], bf16, tag="vt", bufs=4)
            bal.copy(u.vt[:, :, :, d.voff:d.voff + d.span],
                     u.psa[:, :, :, 0:d.span], 4 * d.span)
            slq = out_s[:, 2 * g:2 * g + 2, d.B0:d.B0 + 2, d.w0:d.w1]
            u.q = omp.tile([P, 2, 2, 256], bf16, tag="q", bufs=4)
            bal.tt('sub', u.q[:, :, :, 0:d.Wr], slq,
                   u.om[:, :, :, d.w0 - d.wa:d.w0 - d.wa + d.Wr], 4 * d.Wr)
            u.psb = ppb.tile([P, 2, 2, 256], f32, tag="psb", bufs=2)
            for jj in range(2):
                for hb in range(2):
                    for wc in range(2):
                        nc.tensor.matmul(
                            u.psb[:, jj, hb, au:bu],
                            lhsT=u.vt[:, jj, wc, hb * P:(hb + 1) * P],
                            rhs=u.kht[:, u.kho + wc * d.Wr + au:u.kho + wc * d.Wr + bu],
                            start=(wc == 0), stop=(wc == 1))

        def stage3(u):
            d, g = u.d, u.g
            au, bu = d.wlu - d.w0, d.wru - d.w0
            widu = bu - au
            t2 = bshp.tile([P, 2, 2, 256], bf16, tag="t2", bufs=4)
            bal.bsh_mul(u.psb[:, :, :, au:bu], bshp,
                        u.mt[:, u.i, 0:2, 0:2, d.wlu - d.wa:d.wru - d.wa],
                        t2[:, :, :, 0:widu], 4 * widu, [P, 2, 2, 256], bf16)
            osl = out_s[:, 2 * g:2 * g + 2, d.B0:d.B0 + 2, d.wlu:d.wru]
            bal.tt('add', osl, u.q[:, :, :, au:bu], t2[:, :, :, 0:widu],
                   4 * widu)
            if g == NG - 1:
                for (blk, pi, ws, we) in store_after.get(pos_of[u.dj], []):
                    p0, p1 = pu[blk]
                    for gg in range(NG):
                        nc.sync.dma_start(
                            out=outs_d[(gg, blk, pi)].ap()[:],
                            in_=out_s[p0:p1, 2 * gg:2 * gg + 2, blk, ws:we])

        units = [(dj, g) for dj in order for g in range(NG)]
        ring = []
        for t in range(len(units) + 2):
            if t >= 2:
                stage3(ring[t - 2])
            if t >= 1 and t - 1 < len(units):
                stage2(ring[t - 1])
            if t < len(units):
                ring.append(stage1(*units[t]))
    nc.compile()
    print("balancer loads (us):",
          {k: round(v / 1000, 1) for k, v in bal.load.items()})
    return nc


_CACHE = {}


def _get_program(positions, radius):
    key = (np.asarray(positions, np.float32).tobytes(),
           np.asarray(radius, np.float32).tobytes())
    if key not in _CACHE:
        drops, g0, NB, hmin, hmax = _drop_meta(positions, radius)
        order, level = _topo_order(drops)
        print("emission order:", order, "levels:", level)
        pieces = _store_pieces(drops, order, NB)
        nc = _build_program(drops, g0, NB, hmin, hmax, order, level, pieces)
        _CACHE[key] = (nc, drops, g0, NB, hmin, hmax, order, pieces)
    return _CACHE[key]


def kernel(img, positions, radius, _want_trace=False, **_kw):
    from concourse.bass_utils import run_bass_kernel_spmd
    img = np.asarray(img, np.float32)
    assert img.shape == (B_TOTAL, C, H, W)
    nc, drops, g0, NB, hmin, hmax, order, pieces = _get_program(positions, radius)

    # pack rows [g0, g0+NB*128) to [p, pair(2), w] per (core, pair, blk), bf16,
    # zero-padded outside [hmin, hmax)
    rows_lo, rows_hi = hmin, hmax
    imgb = np.zeros((N_CORES, IC, NB * P, W), _bf16)
    src = img.reshape(N_CORES, IC, H, W)
    imgb[:, :, rows_lo - g0:rows_hi - g0, :] = src[:, :, rows_lo:rows_hi, :].astype(_bf16)
    packed = np.ascontiguousarray(
        imgb.reshape(N_CORES, IC, NB, P, W).transpose(0, 3, 1, 2, 4))

    chunks = [order[0:2], order[2:4], order[4:]]
    WMAX = 256
    base = {}
    for ci, ch in enumerate(chunks):
        mc = np.zeros((P, len(ch), 2, 2, WMAX), _bf16)
        for i, dj in enumerate(ch):
            d = drops[dj]
            mc[:, i, :, :, 0:d.Wt] = d.m_np
        base[f"mc{ci}"] = mc
        base[f"kvc{ci}"] = np.ascontiguousarray(np.concatenate(
            [drops[dj].kv_np.reshape(P, -1) for dj in ch], axis=1))
        base[f"khc{ci}"] = np.ascontiguousarray(np.concatenate(
            [drops[dj].kh_np.reshape(P, -1) for dj in ch], axis=1))
    in_maps = []
    for i in range(N_CORES):
        mp = dict(base)
        for g in range(NG):
            for blk in range(NB):
                mp[f"i{g}b{blk}"] = np.ascontiguousarray(
                    packed[i][:, 2 * g:2 * g + 2, blk, :])
        in_maps.append(mp)
    res = run_bass_kernel_spmd(nc, in_maps, core_ids=list(range(N_CORES)),
                               trace=_want_trace)
    out = img.copy()
    pu = {blk: (max(0, hmin - (g0 + 128 * blk)),
                min(P, hmax - (g0 + 128 * blk))) for blk in range(NB)}
    for i in range(N_CORES):
        oc = out.reshape(N_CORES, IC, H, W)
        for g in range(NG):
            for blk in range(NB):
                p0, p1 = pu[blk]
                r0 = g0 + 128 * blk + p0
                for pi, (ws, we, _pos) in enumerate(pieces[blk]):
                    blkres = res.results[i][f"o{g}b{blk}p{pi}"]
                    # [Pu, 2, wlen] -> rows r0..r0+Pu
                    oc[i, 2 * g:2 * g + 2, r0:r0 + (p1 - p0), ws:we] = \
                        blkres.transpose(1, 0, 2).astype(np.float32)
    if _want_trace:
        return out, res
    return out


# revision 25
# speedup vs baseline: 1.0858x; 1.0025x over previous
"""Trainium2 Bass kernel for nn_Condensation: 10 sequential masked-Gaussian-blur
composites over a [16,3,768,768] image, data-parallel over 8 NeuronCores.

v4 strategy (per core, 2 images = 6 image-channels):
  - Row-offset block grid (delta chosen so EVERY drop's mask support fits in
    exactly 2 h-blocks of 128 rows). Cuts elementwise/mask/matmul work ~25%
    vs a 0-based grid (where 5 drops straddled 3 blocks) and removes the
    false inter-drop dependencies block padding created.
  - Drops emitted in exact topological order of their true spatial overlap
    DAG (non-overlapping drops commute): 3 levels of 4/4/2 drops in flight
    instead of 5 waves of 2 -> much better engine ILP.
  - State resident in SBUF as bf16 [128, 6, NB, 768]; only the 416 rows any
    drop touches are loaded/stored (partial-partition edge blocks, zero-
    padded loads); host copies untouched rows.
  - Separable blur as two banded-matmul passes on TensorE (bf16, f32 PSUM),
    support-clipped bands; q-trick composite (q = out - om) with per-op
    greedy balancing across Vector/Scalar/GpSimd using trace-calibrated
    costs (V bf16 2x-mode vs PSUM 1x, S copy-only, G slow).
  - Stores split per (block, w-piece) keyed to each piece's LAST writer in
    emission order, so most output DMA drains long before the final drop.
"""
import numpy as np
import ml_dtypes

NUM_DROPS = 10
MIN_R, MAX_R = 60.0, 80.0
BETA = 1.8
BLUR_RADII = [11.3535, 17.9381, 5.7966, 10.8586, 5.5301, 15.9075, 12.3225, 13.4871, 6.6639, 9.5413]


def _ksize(r):
    k = int(2 * r) + 1
    return k + 1 if k % 2 == 0 else k


KSIZES = [_ksize(r) for r in BLUR_RADII]
H = W = 768
B_TOTAL, C = 16, 3
N_CORES = 8
B_LOC = B_TOTAL // N_CORES          # 2 images per core
IC = B_LOC * C                      # 6 image-channels per core
NG = IC // 2                        # 3 pairs of image-channels
P = 128
EPS = 5e-3                          # mask support threshold (error-validated)

_bf16 = ml_dtypes.bfloat16
_fp8 = ml_dtypes.float8_e4m3fn


def _conv_matrix(sigma, ksize, n=768):
    """n x n matrix Kmat with blur_1d(x) = Kmat @ x, matching the reference
    (correlation with normalized gaussian, 'reflect' padding)."""
    half = (ksize - 1) * 0.5
    xs = np.linspace(-half, half, ksize)
    pdf = np.exp(-0.5 * (xs / np.float64(sigma)) ** 2)
    k1 = (pdf / pdf.sum()).astype(np.float32).astype(np.float64)
    pad = ksize // 2
    Kmat = np.zeros((n, n), dtype=np.float64)
    idx = np.arange(n)[:, None] + np.arange(ksize)[None, :] - pad
    idx = np.abs(idx)
    idx = np.where(idx >= n, 2 * n - 2 - idx, idx)
    np.add.at(Kmat, (np.repeat(np.arange(n), ksize), idx.ravel()),
              np.tile(k1, n))
    return Kmat.astype(np.float32)


class _Drop:
    pass


def _drop_meta(positions, radius):
    """Host-side per-drop geometry + tensors (shared across cores) on the
    row-offset block grid."""
    pos = np.clip(np.asarray(positions, np.float32), -1.0, 1.0)
    rad = np.clip(np.asarray(radius, np.float32), MIN_R, MAX_R)
    s = float(np.sqrt((-np.log(EPS)) ** (1.0 / BETA)))
    s2 = s * s

    geo = []
    for j in range(NUM_DROPS):
        x0 = (pos[j, 0] + 1.0) / 2.0 * W
        y0 = (pos[j, 1] + 1.0) / 2.0 * H
        wr = rad[j]
        hr = wr * np.float32(0.8)
        p = KSIZES[j] // 2
        h0 = max(0, int(np.floor(y0 - s * hr))) & ~1
        h1 = min(H, (int(np.ceil(y0 + s * hr)) + 2) & ~1)
        w0 = max(0, int(np.floor(x0 - s * wr))) & ~1
        w1 = min(W, (int(np.ceil(x0 + s * wr)) + 2) & ~1)
        geo.append([h0, h1, w0, w1, p, float(x0), float(y0), float(wr), float(hr)])

    # pick an even grid offset so every drop spans exactly 2 blocks
    delta = None
    for dd_ in range(0, 128, 2):
        if all(((g[0] - dd_) % 128) + (g[1] - g[0]) <= 256 for g in geo):
            delta = dd_
            break
    assert delta is not None, "no 2-block grid offset exists"
    hmin = min(g[0] for g in geo)
    hmax = max(g[1] for g in geo)
    g0 = hmin - ((hmin - delta) % 128)
    NB = -((g0 - hmax) // 128)

    drops = []
    for j in range(NUM_DROPS):
        h0, h1, w0, w1, p, x0, y0, wr, hr = geo[j]
        d = _Drop()
        d.j, d.p = j, p
        d.B0 = (h0 - g0) // 128
        d.HBs = g0 + 128 * d.B0
        assert h1 - d.HBs <= 256 and d.B0 + 2 <= NB
        # cap w so Wt <= 256 (two overlapping 128-col chunks)
        wcap = 256 - 2 * p - 2
        while w1 - w0 > wcap:
            if x0 - w0 > w1 - x0:
                w0 += 2
            else:
                w1 -= 2
        d.h0, d.h1, d.w0, d.w1 = h0, h1, w0, w1
        d.span = h1 - h0
        d.Wr = w1 - w0
        d.voff = h0 - d.HBs
        wa = max(0, w0 - p) & ~1
        wb = min(W, (w1 + p + 1) & ~1)
        d.wa, d.wb = wa, wb
        d.Wt = wb - wa
        assert d.Wt <= 256 and d.span <= 256
        d.WBn = (d.Wt + P - 1) // P
        assert d.WBn == 2
        d.cstarts = [wa, wb - P]

        # pass A bands per k-block: output h' range (relative to h0)
        d.bandsA = []
        for k in range(2):
            a = max(0, d.HBs + P * k - p - h0)
            b = min(d.span, d.HBs + P * (k + 1) + p - h0)
            d.bandsA.append((a, b))

        # per h-block composite w-range [wl, wr) from the ellipse extent
        d.hbw = []
        for hb in range(2):
            ra = max(h0, d.HBs + P * hb)
            rb = min(h1, d.HBs + P * (hb + 1))
            if ra - 1 < y0 < rb:
                dh = 0.0
            else:
                dh = min(abs(ra - y0), abs(rb - 1 - y0))
            half = wr * np.sqrt(max(0.0, s2 - (dh / hr) ** 2))
            wl = max(w0, (int(np.floor(x0 - half)) - 2) & ~1)
            wr_ = min(w1, (int(np.ceil(x0 + half)) + 4) & ~1)
            wr_ = max(wr_, wl + 2)
            d.hbw.append((wl, wr_))
        # union composite window across both h-blocks (mask is zero outside
        # each block's own [wl, wr), so fused ops over the union are exact)
        d.wlu = min(wl for wl, _ in d.hbw)
        d.wru = max(wr_ for _, wr_ in d.hbw)

        # mask over [2 blocks of 128 rows] x [wa:wb], zero outside support
        rows = (d.HBs + np.arange(2 * P, dtype=np.int64)).astype(np.float32)
        dd = (rows[:, None] - y0) ** 2 / hr ** 2 + \
             (np.arange(wa, wb, dtype=np.float32)[None, :] - x0) ** 2 / wr ** 2
        m = np.clip(np.exp(-(dd.astype(np.float32) ** np.float32(BETA)) + np.float32(1e-10)), 0.0, 1.0)
        m = np.where(dd <= np.float32(s2), m, 0.0).astype(np.float32)
        mz = np.zeros_like(m)
        for hb in range(2):
            ra = max(h0, d.HBs + P * hb) - d.HBs
            rb = min(h1, d.HBs + P * (hb + 1)) - d.HBs
            wl, wr_ = d.hbw[hb]
            mz[ra:rb, wl - wa:wr_ - wa] = m[ra:rb, wl - wa:wr_ - wa]
        d.m_np = np.ascontiguousarray(
            mz.reshape(2, P, d.Wt).transpose(1, 0, 2)).astype(_bf16)

        MT = _conv_matrix(BLUR_RADII[j], KSIZES[j]).T    # MT[src, dst]
        kv = np.zeros((P, 2, d.span), np.float32)
        for k in range(2):
            r0 = d.HBs + P * k
            lo = max(0, -r0)
            hi = min(P, H - r0)
            if hi > lo:
                kv[lo:hi, k, :] = MT[r0 + lo:r0 + hi, h0:h1]
        d.kv_np = np.ascontiguousarray(kv.astype(_bf16))
        kh = np.zeros((P, 2, d.Wr), np.float32)
        for wc in range(2):
            c = d.cstarts[wc]
            kh[:, wc, :] = MT[c:c + P, w0:w1]
        # the second w-chunk overlaps the first: zero duplicated rows
        dup = d.cstarts[0] + P - d.cstarts[1]
        if dup > 0:
            kh[:dup, 1, :] = 0.0
        d.kh_np = np.ascontiguousarray(kh.astype(_bf16))
        drops.append(d)
    return drops, g0, NB, hmin, hmax


def _topo_order(drops):
    """Exact dependency DAG on (block-range x w-range) slice overlap;
    emission order = stable topological levels."""
    def _dep(i, j):
        di, dj_ = drops[i], drops[j]
        if abs(di.B0 - dj_.B0) > 1:
            return False
        ri, wi = (di.wa, di.wb), (di.w0, di.w1)
        rj, wj = (dj_.wa, dj_.wb), (dj_.w0, dj_.w1)
        for (a, b) in ((wi, rj), (ri, wj), (wi, wj)):
            if max(a[0], b[0]) < min(a[1], b[1]):
                return True
        return False

    level = [0] * NUM_DROPS
    for j in range(NUM_DROPS):
        for i in range(j):
            if _dep(i, j):
                level[j] = max(level[j], level[i] + 1)
    order = sorted(range(NUM_DROPS), key=lambda j: (level[j], j))
    return order, level


def _store_pieces(drops, order, NB):
    """Per block: split [0,W) into up to 3 w-pieces, each tagged with the
    emission position of its LAST writer (-1 = never written)."""
    pieces = {}
    for blk in range(NB):
        last = np.full(W, -1, np.int64)
        for pos, dj in enumerate(order):
            d = drops[dj]
            if d.B0 <= blk <= d.B0 + 1:
                last[d.w0:d.w1] = pos
        runs = []
        ws = 0
        for x in range(1, W + 1):
            if x == W or last[x] != last[ws]:
                runs.append([ws, x, int(last[ws])])
                ws = x
        # merge small runs / cap count; merged run stores after max(pos)
        def _merge_once():
            k = min(range(len(runs)), key=lambda i: runs[i][1] - runs[i][0])
            if k == 0:
                k2 = 1
            elif k == len(runs) - 1:
                k2 = k - 1
            else:
                k2 = k - 1 if (runs[k - 1][1] - runs[k - 1][0]) < (runs[k + 1][1] - runs[k + 1][0]) else k + 1
            a, b = min(k, k2), max(k, k2)
            runs[a] = [runs[a][0], runs[b][1], max(runs[a][2], runs[b][2])]
            del runs[b]
        while len(runs) > 3 or min(r[1] - r[0] for r in runs) < 96:
            _merge_once()
        # even alignment
        for r in runs:
            r[0] &= ~1
        for i in range(len(runs) - 1):
            runs[i][1] = runs[i + 1][0]
        runs[-1][1] = W
        pieces[blk] = [(r[0], r[1], r[2]) for r in runs]
    return pieces


class _Balancer:
    """Greedy static load-balancer across Vector/Scalar/GpSimd with
    HW-measured per-op costs (ns): V sbuf-bf16 TT ~0.62/elem (2x mode),
    V psum-touching 1.1/elem, S copy 1.15/elem, G TT 2.0/elem.
    S (Activation) can only copy; G cannot touch PSUM."""

    def __init__(self, nc):
        self.nc = nc
        self.load = {'V': 0.0, 'S': 0.0, 'G': 0.0}

    def _pick(self, costs):
        eng, c = min(costs, key=lambda ec: self.load[ec[0]] + ec[1])
        self.load[eng] += c
        return eng

    def tt(self, op, out, a, b, fd):
        costs = [('V', fd * 0.62 + 150), ('G', fd * 2.0 + 220)]
        eng = self._pick(costs)
        e = self.nc.vector if eng == 'V' else self.nc.gpsimd
        getattr(e, 'tensor_' + op)(out, a, b)

    def copy(self, out, src, fd):
        # PSUM f32 -> SBUF (V at 1x psum rate, S activation copy)
        eng = self._pick([('V', fd * 1.1 + 200), ('S', fd * 1.15 + 200)])
        if eng == 'V':
            self.nc.vector.tensor_copy(out, src)
        else:
            self.nc.scalar.copy(out=out, in_=src)

    def bsh_mul(self, psb_sl, bshp, m_sl, t2_sl, fd, shape, dt):
        """t2 = m * psb, either via {S|V} psum-copy + {V|G} bf16 mul, or
        V direct mul from PSUM."""
        cV, cS = fd * 1.1 + 200, fd * 1.15 + 200
        mV, mG = fd * 0.62 + 150, fd * 2.0 + 220
        dV = fd * 1.1 + 200
        best, opt = None, None
        for tag, deltas in [('SV', (('S', cS), ('V', mV))),
                            ('SG', (('S', cS), ('G', mG))),
                            ('VG', (('V', cV), ('G', mG))),
                            ('D', (('V', dV),))]:
            tmp = dict(self.load)
            for e, c in deltas:
                tmp[e] += c
            key = (max(tmp.values()), sum(tmp.values()))
            if best is None or key < best:
                best, opt = key, (tag, deltas)
        tag, deltas = opt
        for e, c in deltas:
            self.load[e] += c
        if tag == 'D':
            self.nc.vector.tensor_mul(t2_sl, m_sl, psb_sl)
        else:
            bsh = bshp.tile(shape, dt, tag="Bs")
            bsh_sl = bsh[:, :, :, 0:psb_sl.shape[-1]]
            if tag[0] == 'S':
                self.nc.scalar.copy(out=bsh_sl, in_=psb_sl)
            else:
                self.nc.vector.tensor_copy(bsh_sl, psb_sl)
            e = self.nc.vector if tag[1] == 'V' else self.nc.gpsimd
            e.tensor_mul(t2_sl, m_sl, bsh_sl)


def _build_program(drops, g0, NB, hmin, hmax, order, lvl, pieces):
    from contextlib import ExitStack
    from concourse import bacc, tile, mybir

    f32 = mybir.dt.float32
    bf16 = mybir.dt.bfloat16
    fp8 = mybir.dt.float8e4

    nc = bacc.Bacc("TRN2", target_bir_lowering=False, debug=False,
                   num_devices=N_CORES)

    # input params: per (pair, block-half), always 128 partitions (host
    # zero-pads rows outside [hmin, hmax)); fat DMAs = few triggers
    imgs_d = [[nc.declare_dram_parameter(f"i{g}h{hh}", [P, 2, 2, W], bf16, False)
               for hh in range(2)] for g in range(NG)]
    # output params: per (blk, piece) across ALL channels
    pu = {blk: (max(0, hmin - (g0 + 128 * blk)),
                min(P, hmax - (g0 + 128 * blk))) for blk in range(NB)}
    outs_d = {}
    for blk in range(NB):
        p0, p1 = pu[blk]
        for pi, (ws, we, _pos) in enumerate(pieces[blk]):
            outs_d[(blk, pi)] = nc.declare_dram_parameter(
                f"ob{blk}p{pi}", [p1 - p0, IC, we - ws], bf16, True)

    # drop params batched into 2 chunks by emission order (hot 2 / rest),
    # masks stored once (no jj duplication; ops broadcast via stride-0 AP)
    chunks = [order[0:2], order[2:]]
    kvoffs, khoffs, moffs = {}, {}, {}
    kvlen = [0] * len(chunks)
    khlen = [0] * len(chunks)
    mlen = [0] * len(chunks)
    for ci, ch in enumerate(chunks):
        for dj in ch:
            d = drops[dj]
            kvoffs[dj] = (ci, kvlen[ci])
            khoffs[dj] = (ci, khlen[ci])
            moffs[dj] = (ci, mlen[ci])
            kvlen[ci] += 2 * d.span
            khlen[ci] += 2 * d.Wr
            mlen[ci] += 2 * d.Wt
    pchunks = []
    for ci, ch in enumerate(chunks):
        pchunks.append((
            nc.declare_dram_parameter(f"mc{ci}", [P, mlen[ci]], bf16, False),
            nc.declare_dram_parameter(f"kvc{ci}", [P, kvlen[ci]], bf16, False),
            nc.declare_dram_parameter(f"khc{ci}", [P, khlen[ci]], bf16, False)))

    bal = _Balancer(nc)

    with tile.TileContext(nc) as tc, ExitStack() as ctx:
        outp = ctx.enter_context(tc.tile_pool(name="out_state", bufs=1))
        out_s = outp.tile([P, IC, NB, W], bf16, name="state", tag="state")
        dp = ctx.enter_context(tc.tile_pool(name="dropin", bufs=1))
        omp = ctx.enter_context(tc.tile_pool(name="omq", bufs=8))
        vtp = ctx.enter_context(tc.tile_pool(name="vts", bufs=8))
        bshp = ctx.enter_context(tc.tile_pool(name="bsh", bufs=8))
        ppa = ctx.enter_context(tc.tile_pool(name="psa", bufs=2, space="PSUM"))
        ppb = ctx.enter_context(tc.tile_pool(name="psb", bufs=2, space="PSUM"))

        # ---- PE warm-up: matmuls on a zeroed tile span the load window
        wt = dp.tile([P, 512], bf16, tag="warm")
        nc.gpsimd.memset(wt[:], 0)
        warm = ppa.tile([P, 2, 2, 256], f32, tag="psa")
        for i in range(22):
            nc.tensor.matmul(warm[:, 0, 0, 0:256], lhsT=wt[:, 0:P],
                             rhs=wt[:, 0:256], start=True, stop=True)
        # pre-zero the vt ring so pass-B stationaries never read NaN garbage
        for i in range(4):
            v0 = vtp.tile([P, 2, 2, 256], bf16, tag="vt", bufs=4)
            (nc.vector if i % 2 else nc.gpsimd).memset(v0[:], 0)

        # ---- loads: params chunk0 on scalar; imgs pair-major on sync so
        # each drop chain starts as its blocks arrive; later chunks follow
        ptiles = []
        for ci, ch in enumerate(chunks):
            ptiles.append((
                dp.tile([P, mlen[ci]], bf16, tag=f"mc{ci}", name=f"mc{ci}"),
                dp.tile([P, kvlen[ci]], bf16, tag=f"kvc{ci}", name=f"kvc{ci}"),
                dp.tile([P, khlen[ci]], bf16, tag=f"khc{ci}", name=f"khc{ci}")))
        for t, pd in zip(ptiles[0], pchunks[0]):
            nc.scalar.dma_start(out=t[:], in_=pd.ap()[:])
        # img halves in emission-friendly order: all pairs' lower blocks first
        for hh in range(2):
            for g in range(NG):
                nc.sync.dma_start(
                    out=out_s[:, 2 * g:2 * g + 2, 2 * hh:2 * hh + 2, :],
                    in_=imgs_d[g][hh].ap()[:])
        for t, pd in zip(ptiles[1], pchunks[1]):
            nc.scalar.dma_start(out=t[:], in_=pd.ap()[:])

        # position of each drop in emission order, for store scheduling
        pos_of = {dj: pos for pos, dj in enumerate(order)}
        store_after = {}
        for blk in range(NB):
            for pi, (ws, we, pos) in enumerate(pieces[blk]):
                store_after.setdefault(max(pos, 0), []).append((blk, pi, ws, we))

        # ---- drops: software-pipelined at (drop, pair) granularity so no
        # engine FIFO blocks at its head and PSUM rings (2 bufs each) are
        # recycled only after their reader is emitted.
        #   iteration t: comp(u[t-2]) -> evict+q+passB(u[t-1]) -> om+passA(u[t])
        waves = {}
        for dj in order:
            waves.setdefault(lvl[dj], []).append(dj)

        class _U:
            pass

        def stage1(dj, g):
            u = _U()
            d = drops[dj]
            ci = next(ii for ii, ch in enumerate(chunks) if dj in ch)
            u.d, u.g, u.dj = d, g, dj
            u.i = chunks[ci].index(dj)
            u.mt, u.kvt, u.kht = ptiles[ci]
            _, u.kvo = kvoffs[dj]
            _, u.kho = khoffs[dj]
            _, mo = moffs[dj]
            # mask [P, 2(hb), Wt] stored once; broadcast over jj via stride-0
            u.m2 = u.mt[:, mo:mo + 2 * d.Wt].rearrange(
                "p (k w) -> p k w", k=2)
            sl = out_s[:, 2 * g:2 * g + 2, d.B0:d.B0 + 2, d.wa:d.wb]
            u.om = omp.tile([P, 2, 2, 256], bf16, tag="om", bufs=4)
            bal.tt('mul', u.om[:, :, :, 0:d.Wt],
                   u.m2.unsqueeze(1).broadcast_to([P, 2, 2, d.Wt]), sl,
                   4 * d.Wt)
            u.psa = ppa.tile([P, 2, 2, 256], f32, tag="psa", bufs=2)
            for wc in range(2):
                coff = d.cstarts[wc] - d.wa
                for jj in range(2):
                    for k in range(2):
                        a, b = d.bandsA[k]
                        nc.tensor.matmul(
                            u.psa[:, jj, wc, a:b],
                            lhsT=u.om[:, jj, k, coff:coff + P],
                            rhs=u.kvt[:, u.kvo + k * d.span + a:u.kvo + k * d.span + b],
                            start=(k == 0), stop=(k == 1))
            return u

        def stage2(u):
            d, g = u.d, u.g
            au, bu = d.wlu - d.w0, d.wru - d.w0
            u.vt = vtp.tile([P, 2, 2, 256], bf16, tag="vt", bufs=4)
            bal.copy(u.vt[:, :, :, d.voff:d.voff + d.span],
                     u.psa[:, :, :, 0:d.span], 4 * d.span)
            slq = out_s[:, 2 * g:2 * g + 2, d.B0:d.B0 + 2, d.w0:d.w1]
            u.q = omp.tile([P, 2, 2, 256], bf16, tag="q", bufs=4)
            bal.tt('sub', u.q[:, :, :, 0:d.Wr], slq,
                   u.om[:, :, :, d.w0 - d.wa:d.w0 - d.wa + d.Wr], 4 * d.Wr)
            u.psb = ppb.tile([P, 2, 2, 256], f32, tag="psb", bufs=2)
            for jj in range(2):
                for hb in range(2):
                    for wc in range(2):
                        nc.tensor.matmul(
                            u.psb[:, jj, hb, au:bu],
                            lhsT=u.vt[:, jj, wc, hb * P:(hb + 1) * P],
                            rhs=u.kht[:, u.kho + wc * d.Wr + au:u.kho + wc * d.Wr + bu],
                            start=(wc == 0), stop=(wc == 1))

        def stage3(u):
            d, g = u.d, u.g
            au, bu = d.wlu - d.w0, d.wru - d.w0
            widu = bu - au
            t2 = bshp.tile([P, 2, 2, 256], bf16, tag="t2", bufs=4)
            bal.bsh_mul(u.psb[:, :, :, au:bu], bshp,
                        u.m2[:, :, d.wlu - d.wa:d.wru - d.wa]
                           .unsqueeze(1).broadcast_to([P, 2, 2, widu]),
                        t2[:, :, :, 0:widu], 4 * widu, [P, 2, 2, 256], bf16)
            osl = out_s[:, 2 * g:2 * g + 2, d.B0:d.B0 + 2, d.wlu:d.wru]
            bal.tt('add', osl, u.q[:, :, :, au:bu], t2[:, :, :, 0:widu],
                   4 * widu)
            if g == NG - 1:
                for (blk, pi, ws, we) in store_after.get(pos_of[u.dj], []):
                    p0, p1 = pu[blk]
                    nc.sync.dma_start(
                        out=outs_d[(blk, pi)].ap()[:],
                        in_=out_s[p0:p1, :, blk, ws:we])

        units = [(dj, g) for dj in order for g in range(NG)]
        ring = []
        for t in range(len(units) + 2):
            if t >= 2:
                stage3(ring[t - 2])
            if t >= 1 and t - 1 < len(units):
                stage2(ring[t - 1])
            if t < len(units):
                ring.append(stage1(*units[t]))
    nc.compile()
    print("balancer loads (us):",
          {k: round(v / 1000, 1) for k, v in bal.load.items()})
    return nc


_CACHE = {}


def _get_program(positions, radius):
    key = (np.asarray(positions, np.float32).tobytes(),
           np.asarray(radius, np.float32).tobytes())
    if key not in _CACHE:
        drops, g0, NB, hmin, hmax = _drop_meta(positions, radius)
        order, level = _topo_order(drops)
        print("emission order:", order, "levels:", level)
        pieces = _store_pieces(drops, order, NB)
        nc = _build_program(drops, g0, NB, hmin, hmax, order, level, pieces)
        _CACHE[key] = (nc, drops, g0, NB, hmin, hmax, order, pieces)
    return _CACHE[key]


def kernel(img, positions, radius, _want_trace=False, **_kw):
    from concourse.bass_utils import run_bass_kernel_spmd
    img = np.asarray(img, np.float32)
    assert img.shape == (B_TOTAL, C, H, W)
    nc, drops, g0, NB, hmin, hmax, order, pieces = _get_program(positions, radius)

    # pack rows [g0, g0+NB*128) to [p, pair(2), w] per (core, pair, blk), bf16,
    # zero-padded outside [hmin, hmax)
    rows_lo, rows_hi = hmin, hmax
    imgb = np.zeros((N_CORES, IC, NB * P, W), _bf16)
    src = img.reshape(N_CORES, IC, H, W)
    imgb[:, :, rows_lo - g0:rows_hi - g0, :] = src[:, :, rows_lo:rows_hi, :].astype(_bf16)
    packed = np.ascontiguousarray(
        imgb.reshape(N_CORES, IC, NB, P, W).transpose(0, 3, 1, 2, 4))

    chunks = [order[0:2], order[2:]]
    base = {}
    for ci, ch in enumerate(chunks):
        base[f"mc{ci}"] = np.ascontiguousarray(np.concatenate(
            [drops[dj].m_np.reshape(P, -1) for dj in ch], axis=1))
        base[f"kvc{ci}"] = np.ascontiguousarray(np.concatenate(
            [drops[dj].kv_np.reshape(P, -1) for dj in ch], axis=1))
        base[f"khc{ci}"] = np.ascontiguousarray(np.concatenate(
            [drops[dj].kh_np.reshape(P, -1) for dj in ch], axis=1))
    in_maps = []
    for i in range(N_CORES):
        mp = dict(base)
        for g in range(NG):
            for hh in range(2):
                mp[f"i{g}h{hh}"] = np.ascontiguousarray(
                    packed[i][:, 2 * g:2 * g + 2, 2 * hh:2 * hh + 2, :])
        in_maps.append(mp)
    res = run_bass_kernel_spmd(nc, in_maps, core_ids=list(range(N_CORES)),
                               trace=_want_trace)
    out = img.copy()
    pu = {blk: (max(0, hmin - (g0 + 128 * blk)),
                min(P, hmax - (g0 + 128 * blk))) for blk in range(NB)}
    for i in range(N_CORES):
        oc = out.reshape(N_CORES, IC, H, W)
        for blk in range(NB):
            p0, p1 = pu[blk]
            r0 = g0 + 128 * blk + p0
            for pi, (ws, we, _pos) in enumerate(pieces[blk]):
                blkres = res.results[i][f"ob{blk}p{pi}"]
                # [Pu, IC, wlen] -> rows r0..r0+Pu
                oc[i, :, r0:r0 + (p1 - p0), ws:we] = \
                    blkres.transpose(1, 0, 2).astype(np.float32)
    if _want_trace:
        return out, res
    return out
